# revision 41
# baseline (speedup 1.0000x reference)
"""Trainium2 Bass kernel for BinConv2d:
   y = relu(conv2d(sign(batchnorm_train(x)), W, pad=1) + b)

Sharding: data-parallel over batch, 4 images per core on 8 cores.

Single SPMD launch: per-core BN partial sums (DVE reduce + ACT square) are
combined across cores with a tiny [128,2] AllReduce; sign() needs only a
per-channel affine threshold (sign(gamma*x + (beta*sigma - gamma*mean))),
so the variance path never touches per-element math.

Conv is 9 "taps" of a 64->64 matmul over all pixels. Binarized activations
(exact +-1 in fp16) are stored zero-padded [64ch, 114*114] per image, plus
a row-shifted duplicate on partitions 64..127 so taps (kh,kw) and (kh+1,kw)
pair into one K=128 matmul. Two 4-row output chunks run concurrently on
the two column halves of the PE array via tile_position.

The conv is rhs-stream-bound (~1 column/cycle into the PE); a short
full-array warm-up burst gated on the AllReduce result covers the HAM
clock ramp before the conv stream starts. A sacrificial AllReduce issued
at t~0 absorbs the inter-core rendezvous barrier and the ncfw wake-up
so the real AllReduce starts immediately once stats are ready.

Image order 0,2,1,3 keeps at most 3 binarized images resident (SBUF).
"""

import sys
from contextlib import ExitStack

import numpy as np

try:
    import concourse.bass as bass  # noqa: F401
except ImportError:  # pragma: no cover
    sys.path.insert(0, "/opt/trn_rl_repo")
    import concourse.bass as bass  # noqa: F401

import concourse.bacc as bacc
import concourse.tile as tile
from concourse import mybir
from concourse.bass_utils import run_bass_kernel_spmd
from concourse.masks import make_identity

F32 = mybir.dt.float32
WDT = mybir.dt.float16  # dtype for conv weights and binarized activations

N_CORES = 8
N_IMG = 4  # images per core (batch 32 / 8 cores)
NHALF = N_IMG // 2
C = 64
H = 112
W = 112
HP = H + 2  # 114
WP = W + 2  # 114
IMG = HP * WP  # 12996
EPS = 1e-4

PIX = H * W
Q_ROWS = 28  # rows per x load chunk
NQ = H // Q_ROWS  # 4
QW = Q_ROWS * W  # 3136
ROWS_PER_CHUNK = 4  # output rows per matmul chunk (N = 4*112 = 448)
NMM = ROWS_PER_CHUNK * W  # 448
N_SLOTS = H // (2 * ROWS_PER_CHUNK)  # 14

N_WARM_POST = 8  # warm-up burst gated on the AllReduce result


def build_program(n_cores=N_CORES, n_img=N_IMG):
    assert n_img % 2 == 0
    nhalf = n_img // 2
    fpart = nhalf * PIX

    nc = bacc.Bacc(
        "TRN2", target_bir_lowering=False, debug=False, num_devices=n_cores
    )
    x = nc.dram_tensor("x", [n_img, C, H, W], F32, kind="ExternalInput")
    gamma = nc.dram_tensor("gamma", [C], F32, kind="ExternalInput")
    beta = nc.dram_tensor("beta", [C], F32, kind="ExternalInput")
    Wt = nc.dram_tensor("W", [C, C, 3, 3], F32, kind="ExternalInput")
    bt = nc.dram_tensor("b", [C], F32, kind="ExternalInput")
    y = nc.dram_tensor("y", [n_img, C, H, W], F32, kind="ExternalOutput")

    with tile.TileContext(nc) as tc, ExitStack() as ctx:
        const = ctx.enter_context(tc.tile_pool(name="const", bufs=1))
        bigp = ctx.enter_context(tc.tile_pool(name="big", bufs=1))
        xbp = ctx.enter_context(tc.tile_pool(name="xb", bufs=3))
        statp = ctx.enter_context(tc.tile_pool(name="stat", bufs=1))
        psump = ctx.enter_context(tc.tile_pool(name="ps", bufs=3, space="PSUM"))
        psdum = ctx.enter_context(tc.tile_pool(name="psd", bufs=3, space="PSUM"))
        pstr = ctx.enter_context(tc.tile_pool(name="pst", bufs=2, space="PSUM"))
        outp = ctx.enter_context(tc.tile_pool(name="out", bufs=3))
        dramp = ctx.enter_context(tc.tile_pool(name="dram", bufs=1, space="DRAM"))

        # ---- constants / dummies ----
        wdum = const.tile([128, C], F32)
        nc.gpsimd.memset(wdum, 1.0)
        wdum2 = const.tile([128, C], F32)
        nc.gpsimd.memset(wdum2, 1.0)
        identity64 = const.tile([C, C], F32)
        make_identity(nc, identity64)
        eps64 = const.tile([C, 1], F32)
        nc.gpsimd.memset(eps64, EPS)

        xsb = bigp.tile([128, fpart], F32)
        xsb_v = xsb.rearrange("p (n2 h w) -> p n2 h w", n2=nhalf, h=H)

        dum_i = 0

        def dummy_mm(rhs_base, lhsT=None):
            nonlocal dum_i
            psD = psdum.tile([C, NMM], F32, tag="psd")
            nc.tensor.matmul(
                psD,
                wdum if lhsT is None else lhsT,
                xsb[:, rhs_base : rhs_base + NMM],
                start=True,
                stop=True,
                skip_group_check=True,
            )
            dum_i += 1

        # ---- all x load triggers up front (descriptor-cheap APs):
        # 16 DMAs of [64ch, 28*112 contiguous], alternating sync/scalar ----
        for n2 in range(nhalf):
            for q in range(NQ):
                base = n2 * PIX + q * QW
                for half in range(2):
                    n = half * nhalf + n2
                    dst = xsb[half * C : half * C + C, base : base + QW]
                    eng = nc.sync
                    eng.dma_start(
                        out=dst.rearrange("c (h w) -> c h w", w=W),
                        in_=x.ap()[n, :, q * Q_ROWS : (q + 1) * Q_ROWS, :],
                    )
        # const DMAs on gpsimd (won't block the load queues)
        wsb = const.tile([C, C, 9], F32)
        nc.gpsimd.dma_start(
            out=wsb, in_=Wt.ap().rearrange("o c kh kw -> o c (kh kw)")
        )
        b2 = const.tile([128, 1], F32)
        bsrc = bt.ap().rearrange("(c u) -> c u", u=1)
        nc.gpsimd.dma_start(out=b2[0:C, :], in_=bsrc)
        nc.gpsimd.dma_start(out=b2[C:128, :], in_=bsrc)
        gamma2 = const.tile([128, 1], F32)
        gsrc = gamma.ap().rearrange("(c u) -> c u", u=1)
        nc.gpsimd.dma_start(out=gamma2[0:C, :], in_=gsrc)
        nc.gpsimd.dma_start(out=gamma2[C:128, :], in_=gsrc)
        beta64 = const.tile([C, 1], F32)
        nc.gpsimd.dma_start(
            out=beta64, in_=beta.ap().rearrange("(c u) -> c u", u=1)
        )

        # ---- per-chunk BN partials (DVE sum, ACT sum-of-squares) ----
        n_chunks = nhalf * NQ
        sums = statp.tile([128, n_chunks], F32)
        sqs = statp.tile([128, n_chunks], F32)
        sqscr = statp.tile([128, QW], F32)
        for idx in range(n_chunks):
            base = idx * QW  # (n2, q) in row-major == contiguous slices
            nc.vector.tensor_reduce(
                out=sums[:, idx : idx + 1],
                in_=xsb[:, base : base + QW],
                axis=mybir.AxisListType.X,
                op=mybir.AluOpType.add,
            )
            nc.scalar.activation(
                out=sqscr,
                in_=xsb[:, base : base + QW],
                func=mybir.ActivationFunctionType.Square,
                accum_out=sqs[:, idx : idx + 1],
            )

        # fp16 weight views while stats run: w2[0:64,t,:] = tap t,
        # w2[64:128,t,:] = tap t+3 (PE transposes produce lhsT[c,o])
        w2 = const.tile([128, 9, C], WDT)
        for t in range(9):
            psT = pstr.tile([C, C], F32, tag="pst")
            nc.tensor.transpose(psT, wsb[:, :, t], identity64)
            nc.scalar.activation(
                out=w2[0:C, t, :], in_=psT,
                func=mybir.ActivationFunctionType.Copy,
            )
            if t >= 3:
                nc.scalar.activation(
                    out=w2[C:128, t - 3, :], in_=psT,
                    func=mybir.ActivationFunctionType.Copy,
                )

        # ---- AllReduce of (sum x, sum x^2) ----
        arin = statp.tile([128, 2], F32)
        nc.vector.tensor_reduce(
            out=arin[:, 0:1], in_=sums,
            axis=mybir.AxisListType.X, op=mybir.AluOpType.add,
        )
        nc.vector.tensor_reduce(
            out=arin[:, 1:2], in_=sqs,
            axis=mybir.AxisListType.X, op=mybir.AluOpType.add,
        )
        cc_in = dramp.tile([128, 2], F32)
        cc_out = dramp.tile([128, 2], F32)
        nc.sync.dma_start(out=cc_in, in_=arin)
        if n_cores > 1:
            nc.gpsimd.collective_compute(
                "AllReduce",
                mybir.AluOpType.add,
                replica_groups=[list(range(n_cores))],
                ins=[cc_in[:].opt()],
                outs=[cc_out[:].opt()],
            )
        else:
            nc.gpsimd.dma_start(out=cc_out, in_=cc_in)
        ar = statp.tile([128, 2], F32)
        nc.sync.dma_start(out=ar, in_=cc_out)
        # post-AR warm-up matmuls gate on wdum2, which depends on ar
        nc.vector.tensor_scalar_mul(wdum2[:, 0:2], ar, 0.0)

        # ---- fold -> per-channel threshold: d = beta*sigma - gamma*mean
        total_count = n_cores * 2 * fpart
        hi = statp.tile([C, 2], F32)
        nc.scalar.activation(
            out=hi, in_=ar[C:128, :], func=mybir.ActivationFunctionType.Copy
        )
        tot = statp.tile([C, 2], F32)
        nc.vector.tensor_add(out=tot, in0=ar[0:C, :], in1=hi)
        mean64 = statp.tile([C, 1], F32)
        nc.vector.tensor_scalar_mul(mean64, tot[:, 0:1], 1.0 / total_count)
        e2 = statp.tile([C, 1], F32)
        nc.vector.tensor_scalar_mul(e2, tot[:, 1:2], 1.0 / total_count)
        var64 = statp.tile([C, 1], F32)
        nc.vector.tensor_mul(out=var64, in0=mean64, in1=mean64)
        nc.vector.tensor_sub(out=var64, in0=e2, in1=var64)
        sigma = statp.tile([C, 1], F32)
        nc.scalar.activation(
            out=sigma, in_=var64,
            func=mybir.ActivationFunctionType.Sqrt, bias=eps64,
        )
        d64 = statp.tile([C, 1], F32)
        nc.vector.tensor_mul(out=d64, in0=beta64, in1=sigma)
        t2 = statp.tile([C, 1], F32)
        nc.vector.tensor_mul(out=t2, in0=gamma2[0:C, :], in1=mean64)
        nc.vector.tensor_sub(out=d64, in0=d64, in1=t2)
        d2 = statp.tile([128, 1], F32)
        nc.vector.tensor_copy(out=d2[0:C, :], in_=d64)
        nc.scalar.activation(
            out=d2[C:128, :], in_=d64, func=mybir.ActivationFunctionType.Copy
        )

        # PE warm-up burst while the threshold fold + first binarize run
        for i in range(N_WARM_POST):
            dummy_mm((i % (fpart // NMM)) * NMM, lhsT=wdum2)

        # ---- conv slot ----
        out_engines = (nc.sync, nc.scalar)
        out_dma_i = 0

        def conv_slot(n, xbv, s):
            nonlocal out_dma_i
            h0 = s * 2 * ROWS_PER_CHUNK
            h1 = h0 + ROWS_PER_CHUNK
            P = psump.tile([128, NMM], F32, tag="psum")
            mms = []
            for kw in range(3):
                for cg, hb in ((0, h0), (64, h1)):
                    mms.append((cg, hb, kw, True))
            for kw in range(3):
                for cg, hb in ((0, h0), (64, h1)):
                    mms.append((cg, hb, kw, False))
            cg_seen = set()
            cg_last = {cg: max(i for i, m in enumerate(mms) if m[0] == cg)
                       for cg in (0, 64)}
            for i, (cg, hb, kw, is_pair) in enumerate(mms):
                if is_pair:
                    lhsT = w2[:, kw, :]
                    rhs = xbv[:, hb : hb + ROWS_PER_CHUNK, kw : kw + W]
                else:
                    lhsT = w2[0:C, 6 + kw, :]
                    rhs = xbv[0:C, hb + 2 : hb + 2 + ROWS_PER_CHUNK, kw : kw + W]
                nc.tensor.matmul(
                    P[cg : cg + C, :],
                    lhsT,
                    rhs,
                    start=(cg not in cg_seen),
                    stop=(i == cg_last[cg]),
                    tile_position=(0, cg),
                    skip_group_check=True,
                )
                cg_seen.add(cg)
            # epilogue relu(P + b): alternate ACT/DVE
            osb = outp.tile([128, NMM], F32, tag="osb")
            if s % 2 == 0:
                nc.scalar.activation(
                    out=osb, in_=P,
                    func=mybir.ActivationFunctionType.Relu, bias=b2,
                )
            else:
                nc.vector.tensor_scalar(
                    out=osb, in0=P, scalar1=b2, scalar2=0.0,
                    op0=mybir.AluOpType.add, op1=mybir.AluOpType.max,
                )
            ov = osb.rearrange("p (h w) -> p h w", w=W)
            e0 = out_engines[out_dma_i % 2]
            e1 = out_engines[(out_dma_i + 1) % 2]
            out_dma_i += 2
            e0.dma_start(
                out=y.ap()[n, :, h0 : h0 + ROWS_PER_CHUNK, :],
                in_=ov[0:C, :, :],
            )
            e1.dma_start(
                out=y.ap()[n, :, h1 : h1 + ROWS_PER_CHUNK, :],
                in_=ov[C:128, :, :],
            )

        # ---- per image (order 0,2,1,3 to cap live xb tiles at 3):
        # binarize in 2 row-chunks then conv ----
        h_split = 56
        img_order = []
        for n2 in range(nhalf):
            img_order += [n2, nhalf + n2]
        for n in img_order:
            half = n // nhalf
            n2 = n % nhalf
            xbt = xbp.tile([128, IMG], WDT, tag="xb")
            xbv = xbt.rearrange("p (hp wp) -> p hp wp", wp=WP)
            nc.gpsimd.memset(xbv[0:C, 0:1, :], 0.0)
            nc.gpsimd.memset(xbv[0:C, HP - 1 : HP, :], 0.0)
            nc.gpsimd.memset(xbv[0:C, 1 : HP - 1, 0:1], 0.0)
            nc.gpsimd.memset(xbv[0:C, 1 : HP - 1, WP - 1 : WP], 0.0)
            for ci, (h0c, h1c) in enumerate(((0, h_split), (h_split, H))):
                nc.scalar.activation(
                    out=xbv[0:C, 1 + h0c : 1 + h1c, 1 : WP - 1],
                    in_=xsb_v[half * C : half * C + C, n2, h0c:h1c, :],
                    func=mybir.ActivationFunctionType.Sign,
                    scale=gamma2[half * C : half * C + C, :],
                    bias=d2[half * C : half * C + C, :],
                )
                # copy B rows = A rows + 1 (chunked, no holes; last chunk
                # runs through row 112 -- A row 113 is the zero border)
                lo = 0 if ci == 0 else (h0c - 1) * WP
                hi_ = (h1c - 1) * WP if h1c < H else IMG - WP
                nc.vector.tensor_copy(
                    out=xbt[C:128, lo:hi_], in_=xbt[0:C, lo + WP : hi_ + WP]
                )
                s_lo = 0 if ci == 0 else (h_split - 9) // 8 + 1
                s_hi = (h1c - 9) // 8 if h1c < H else N_SLOTS - 1
                for s in range(s_lo, s_hi + 1):
                    conv_slot(n, xbv, s)

    nc.compile()
    return nc


_CACHE = {}


def _get_program(n_cores=N_CORES, n_img=N_IMG):
    key = (n_cores, n_img)
    if key not in _CACHE:
        _CACHE[key] = build_program(n_cores, n_img)
    return _CACHE[key]


def kernel(x, gamma, beta, W, b, _trace=False):
    x = np.ascontiguousarray(x, dtype=np.float32)
    assert x.shape[0] == N_CORES * N_IMG, x.shape
    nc = _get_program(N_CORES, N_IMG)
    in_maps = []
    for c in range(N_CORES):
        in_maps.append(
            {
                "x": x[c * N_IMG : (c + 1) * N_IMG],
                "gamma": np.ascontiguousarray(gamma, np.float32),
                "beta": np.ascontiguousarray(beta, np.float32),
                "W": np.ascontiguousarray(W, np.float32),
                "b": np.ascontiguousarray(b, np.float32),
            }
        )
    res = run_bass_kernel_spmd(
        nc, in_maps, core_ids=list(range(N_CORES)), trace=_trace
    )
    out = np.concatenate([res.results[c]["y"] for c in range(N_CORES)], axis=0)
    if _trace:
        kernel._last_result = res
    return out


# revision 44
# speedup vs baseline: 1.0563x; 1.0563x over previous
"""Trainium2 Bass kernel for BinConv2d:
   y = relu(conv2d(sign(batchnorm_train(x)), W, pad=1) + b)

Sharding: data-parallel over batch, 4 images per core on 8 cores.

Single SPMD launch: per-core BN partial sums (DVE reduce + ACT square) are
combined across cores with a tiny [128,2] AllReduce; sign() needs only a
per-channel affine threshold (sign(gamma*x + (beta*sigma - gamma*mean))),
so the variance path never touches per-element math.

Conv is 9 "taps" of a 64->64 matmul over all pixels. Binarized activations
(exact +-1 in fp16) are stored zero-padded [64ch, 114*114] per image, plus
a row-shifted duplicate on partitions 64..127 so taps (kh,kw) and (kh+1,kw)
pair into one K=128 matmul. Two 4-row output chunks run concurrently on
the two column halves of the PE array via tile_position.

The conv is rhs-stream-bound (~1 column/cycle into the PE); a short
full-array warm-up burst gated on the AllReduce result covers the HAM
clock ramp before the conv stream starts. A sacrificial AllReduce issued
at t~0 absorbs the inter-core rendezvous barrier and the ncfw wake-up
so the real AllReduce starts immediately once stats are ready.

Image order 0,2,1,3 keeps at most 3 binarized images resident (SBUF).
"""

import sys
from contextlib import ExitStack

import numpy as np

try:
    import concourse.bass as bass  # noqa: F401
except ImportError:  # pragma: no cover
    sys.path.insert(0, "/opt/trn_rl_repo")
    import concourse.bass as bass  # noqa: F401

import concourse.bacc as bacc
import concourse.tile as tile
from concourse import mybir
from concourse.bass_utils import run_bass_kernel_spmd
from concourse.masks import make_identity

F32 = mybir.dt.float32
WDT = mybir.dt.float16  # dtype for conv weights and binarized activations

N_CORES = 8
N_IMG = 4  # images per core (batch 32 / 8 cores)
NHALF = N_IMG // 2
C = 64
H = 112
W = 112
HP = H + 2  # 114
WP = W + 2  # 114
IMG = HP * WP  # 12996
EPS = 1e-4

PIX = H * W
Q_ROWS = 28  # rows per x load chunk
NQ = H // Q_ROWS  # 4
QW = Q_ROWS * W  # 3136
ROWS_PER_CHUNK = 4  # output rows per matmul chunk (N = 4*112 = 448)
NMM = ROWS_PER_CHUNK * W  # 448
N_SLOTS = H // (2 * ROWS_PER_CHUNK)  # 14

N_WARM_POST = 8  # warm-up burst gated on the AllReduce result


def build_program(n_cores=N_CORES, n_img=N_IMG):
    assert n_img % 2 == 0
    nhalf = n_img // 2
    fpart = nhalf * PIX

    nc = bacc.Bacc(
        "TRN2", target_bir_lowering=False, debug=False, num_devices=n_cores
    )
    x = nc.dram_tensor("x", [n_img, C, H, W], F32, kind="ExternalInput")
    gamma = nc.dram_tensor("gamma", [C], F32, kind="ExternalInput")
    beta = nc.dram_tensor("beta", [C], F32, kind="ExternalInput")
    Wt = nc.dram_tensor("W", [C, C, 3, 3], F32, kind="ExternalInput")
    bt = nc.dram_tensor("b", [C], F32, kind="ExternalInput")
    y = nc.dram_tensor("y", [n_img, C, H, W], F32, kind="ExternalOutput")

    with tile.TileContext(nc) as tc, ExitStack() as ctx:
        const = ctx.enter_context(tc.tile_pool(name="const", bufs=1))
        bigp = ctx.enter_context(tc.tile_pool(name="big", bufs=1))
        xbp = ctx.enter_context(tc.tile_pool(name="xb", bufs=3))
        statp = ctx.enter_context(tc.tile_pool(name="stat", bufs=1))
        psump = ctx.enter_context(tc.tile_pool(name="ps", bufs=3, space="PSUM"))
        psdum = ctx.enter_context(tc.tile_pool(name="psd", bufs=3, space="PSUM"))
        pstr = ctx.enter_context(tc.tile_pool(name="pst", bufs=2, space="PSUM"))
        outp = ctx.enter_context(tc.tile_pool(name="out", bufs=3))
        dramp = ctx.enter_context(tc.tile_pool(name="dram", bufs=1, space="DRAM"))

        # ---- constants / dummies ----
        wdum = const.tile([128, C], F32)
        nc.gpsimd.memset(wdum, 1.0)
        wdum2 = const.tile([128, C], F32)
        nc.gpsimd.memset(wdum2, 1.0)
        identity64 = const.tile([C, C], F32)
        make_identity(nc, identity64)
        eps64 = const.tile([C, 1], F32)
        nc.gpsimd.memset(eps64, EPS)

        xsb = bigp.tile([128, fpart], F32)
        xsb_v = xsb.rearrange("p (n2 h w) -> p n2 h w", n2=nhalf, h=H)

        dum_i = 0

        def dummy_mm(rhs_base, lhsT=None):
            nonlocal dum_i
            psD = psdum.tile([C, NMM], F32, tag="psd")
            nc.tensor.matmul(
                psD,
                wdum if lhsT is None else lhsT,
                xsb[:, rhs_base : rhs_base + NMM],
                start=True,
                stop=True,
                skip_group_check=True,
            )
            dum_i += 1

        # ---- all x load triggers up front (descriptor-cheap APs):
        # 16 DMAs of [64ch, 28*112 contiguous], alternating sync/scalar ----
        for n2 in range(nhalf):
            for q in range(NQ):
                base = n2 * PIX + q * QW
                for half in range(2):
                    n = half * nhalf + n2
                    dst = xsb[half * C : half * C + C, base : base + QW]
                    eng = nc.sync
                    eng.dma_start(
                        out=dst.rearrange("c (h w) -> c h w", w=W),
                        in_=x.ap()[n, :, q * Q_ROWS : (q + 1) * Q_ROWS, :],
                    )
        # const DMAs on gpsimd (won't block the load queues)
        wsb = const.tile([C, C, 9], F32)
        nc.gpsimd.dma_start(
            out=wsb, in_=Wt.ap().rearrange("o c kh kw -> o c (kh kw)")
        )
        b2 = const.tile([128, 1], F32)
        bsrc = bt.ap().rearrange("(c u) -> c u", u=1)
        nc.gpsimd.dma_start(out=b2[0:C, :], in_=bsrc)
        nc.gpsimd.dma_start(out=b2[C:128, :], in_=bsrc)
        gamma2 = const.tile([128, 1], F32)
        gsrc = gamma.ap().rearrange("(c u) -> c u", u=1)
        nc.gpsimd.dma_start(out=gamma2[0:C, :], in_=gsrc)
        nc.gpsimd.dma_start(out=gamma2[C:128, :], in_=gsrc)
        beta64 = const.tile([C, 1], F32)
        nc.gpsimd.dma_start(
            out=beta64, in_=beta.ap().rearrange("(c u) -> c u", u=1)
        )

        # ---- per-chunk BN partials (DVE sum, ACT sum-of-squares) ----
        n_chunks = nhalf * NQ
        sums = statp.tile([128, n_chunks], F32)
        sqs = statp.tile([128, n_chunks], F32)
        sqscr = statp.tile([128, QW], F32)
        for idx in range(n_chunks):
            base = idx * QW  # (n2, q) in row-major == contiguous slices
            nc.vector.tensor_reduce(
                out=sums[:, idx : idx + 1],
                in_=xsb[:, base : base + QW],
                axis=mybir.AxisListType.X,
                op=mybir.AluOpType.add,
            )
            nc.scalar.activation(
                out=sqscr,
                in_=xsb[:, base : base + QW],
                func=mybir.ActivationFunctionType.Square,
                accum_out=sqs[:, idx : idx + 1],
            )

        # fp16 weight views while stats run: w2[0:64,t,:] = tap t,
        # w2[64:128,t,:] = tap t+3 (PE transposes produce lhsT[c,o])
        w2 = const.tile([128, 9, C], WDT)
        for t in range(9):
            psT = pstr.tile([C, C], F32, tag="pst")
            nc.tensor.transpose(psT, wsb[:, :, t], identity64)
            nc.scalar.activation(
                out=w2[0:C, t, :], in_=psT,
                func=mybir.ActivationFunctionType.Copy,
            )
            if t >= 3:
                nc.scalar.activation(
                    out=w2[C:128, t - 3, :], in_=psT,
                    func=mybir.ActivationFunctionType.Copy,
                )

        # ---- AllReduce of (sum x, sum x^2) ----
        arin = statp.tile([128, 2], F32)
        nc.vector.tensor_reduce(
            out=arin[:, 0:1], in_=sums,
            axis=mybir.AxisListType.X, op=mybir.AluOpType.add,
        )
        nc.vector.tensor_reduce(
            out=arin[:, 1:2], in_=sqs,
            axis=mybir.AxisListType.X, op=mybir.AluOpType.add,
        )
        cc_in = dramp.tile([128, 2], F32)
        cc_out = dramp.tile([128, 2], F32)
        nc.sync.dma_start(out=cc_in, in_=arin)
        if n_cores > 1:
            nc.gpsimd.collective_compute(
                "AllReduce",
                mybir.AluOpType.add,
                replica_groups=[list(range(n_cores))],
                ins=[cc_in[:].opt()],
                outs=[cc_out[:].opt()],
            )
        else:
            nc.gpsimd.dma_start(out=cc_out, in_=cc_in)
        ar = statp.tile([128, 2], F32)
        nc.sync.dma_start(out=ar, in_=cc_out)
        # post-AR warm-up matmuls gate on wdum2, which depends on ar
        nc.vector.tensor_scalar_mul(wdum2[:, 0:2], ar, 0.0)

        # ---- fold -> per-channel threshold: d = beta*sigma - gamma*mean
        total_count = n_cores * 2 * fpart
        hi = statp.tile([C, 2], F32)
        nc.scalar.activation(
            out=hi, in_=ar[C:128, :], func=mybir.ActivationFunctionType.Copy
        )
        tot = statp.tile([C, 2], F32)
        nc.vector.tensor_add(out=tot, in0=ar[0:C, :], in1=hi)
        mean64 = statp.tile([C, 1], F32)
        nc.vector.tensor_scalar_mul(mean64, tot[:, 0:1], 1.0 / total_count)
        e2 = statp.tile([C, 1], F32)
        nc.vector.tensor_scalar_mul(e2, tot[:, 1:2], 1.0 / total_count)
        var64 = statp.tile([C, 1], F32)
        nc.vector.tensor_mul(out=var64, in0=mean64, in1=mean64)
        nc.vector.tensor_sub(out=var64, in0=e2, in1=var64)
        sigma = statp.tile([C, 1], F32)
        nc.scalar.activation(
            out=sigma, in_=var64,
            func=mybir.ActivationFunctionType.Sqrt, bias=eps64,
        )
        d64 = statp.tile([C, 1], F32)
        nc.vector.tensor_mul(out=d64, in0=beta64, in1=sigma)
        t2 = statp.tile([C, 1], F32)
        nc.vector.tensor_mul(out=t2, in0=gamma2[0:C, :], in1=mean64)
        nc.vector.tensor_sub(out=d64, in0=d64, in1=t2)
        d2 = statp.tile([128, 1], F32)
        nc.vector.tensor_copy(out=d2[0:C, :], in_=d64)
        nc.scalar.activation(
            out=d2[C:128, :], in_=d64, func=mybir.ActivationFunctionType.Copy
        )

        # PE warm-up burst while the threshold fold + first binarize run
        for i in range(N_WARM_POST):
            dummy_mm((i % (fpart // NMM)) * NMM, lhsT=wdum2)

        # ---- conv slot ----
        out_engines = (nc.sync, nc.scalar)
        out_dma_i = 0

        def conv_slot(n, xbv, s):
            nonlocal out_dma_i
            h0 = s * 2 * ROWS_PER_CHUNK
            h1 = h0 + ROWS_PER_CHUNK
            P = psump.tile([128, NMM], F32, tag="psum")
            mms = []
            for kw in range(3):
                for cg, hb in ((0, h0), (64, h1)):
                    mms.append((cg, hb, kw, True))
            for kw in range(3):
                for cg, hb in ((0, h0), (64, h1)):
                    mms.append((cg, hb, kw, False))
            cg_seen = set()
            cg_last = {cg: max(i for i, m in enumerate(mms) if m[0] == cg)
                       for cg in (0, 64)}
            for i, (cg, hb, kw, is_pair) in enumerate(mms):
                if is_pair:
                    lhsT = w2[:, kw, :]
                    rhs = xbv[:, hb : hb + ROWS_PER_CHUNK, kw : kw + W]
                else:
                    lhsT = w2[0:C, 6 + kw, :]
                    rhs = xbv[0:C, hb + 2 : hb + 2 + ROWS_PER_CHUNK, kw : kw + W]
                nc.tensor.matmul(
                    P[cg : cg + C, :],
                    lhsT,
                    rhs,
                    start=(cg not in cg_seen),
                    stop=(i == cg_last[cg]),
                    tile_position=(0, cg),
                    skip_group_check=True,
                )
                cg_seen.add(cg)
            # epilogue relu(P + b): alternate ACT/DVE
            osb = outp.tile([128, NMM], F32, tag="osb")
            if s % 2 == 0:
                nc.scalar.activation(
                    out=osb, in_=P,
                    func=mybir.ActivationFunctionType.Relu, bias=b2,
                )
            else:
                nc.vector.tensor_scalar(
                    out=osb, in0=P, scalar1=b2, scalar2=0.0,
                    op0=mybir.AluOpType.add, op1=mybir.AluOpType.max,
                )
            ov = osb.rearrange("p (h w) -> p h w", w=W)
            e0 = out_engines[out_dma_i % 2]
            e1 = out_engines[(out_dma_i + 1) % 2]
            out_dma_i += 2
            e0.dma_start(
                out=y.ap()[n, :, h0 : h0 + ROWS_PER_CHUNK, :],
                in_=ov[0:C, :, :],
            )
            e1.dma_start(
                out=y.ap()[n, :, h1 : h1 + ROWS_PER_CHUNK, :],
                in_=ov[C:128, :, :],
            )

        # ---- per image (order 0,2,1,3 to cap live xb tiles at 3):
        # binarize in 2 row-chunks then conv ----
        h_split = 56
        img_order = []
        for n2 in range(nhalf):
            img_order += [n2, nhalf + n2]
        for n in img_order:
            half = n // nhalf
            n2 = n % nhalf
            xbt = xbp.tile([128, IMG], WDT, tag="xb")
            xbv = xbt.rearrange("p (hp wp) -> p hp wp", wp=WP)
            nc.gpsimd.memset(xbv[0:C, 0:1, :], 0.0)
            nc.gpsimd.memset(xbv[0:C, HP - 1 : HP, :], 0.0)
            nc.gpsimd.memset(xbv[0:C, 1 : HP - 1, 0:1], 0.0)
            nc.gpsimd.memset(xbv[0:C, 1 : HP - 1, WP - 1 : WP], 0.0)
            for ci, (h0c, h1c) in enumerate(((0, h_split), (h_split, H))):
                nc.scalar.activation(
                    out=xbv[0:C, 1 + h0c : 1 + h1c, 1 : WP - 1],
                    in_=xsb_v[half * C : half * C + C, n2, h0c:h1c, :],
                    func=mybir.ActivationFunctionType.Sign,
                    scale=gamma2[half * C : half * C + C, :],
                    bias=d2[half * C : half * C + C, :],
                )
                # copy B rows = A rows + 1 (chunked, no holes; last chunk
                # runs through row 112 -- A row 113 is the zero border)
                lo = 0 if ci == 0 else (h0c - 1) * WP
                hi_ = (h1c - 1) * WP if h1c < H else IMG - WP
                nc.vector.tensor_copy(
                    out=xbt[C:128, lo:hi_], in_=xbt[0:C, lo + WP : hi_ + WP]
                )
                s_lo = 0 if ci == 0 else (h_split - 9) // 8 + 1
                s_hi = (h1c - 9) // 8 if h1c < H else N_SLOTS - 1
                for s in range(s_lo, s_hi + 1):
                    conv_slot(n, xbv, s)

    nc.compile()
    return nc


_CACHE = {}


def _get_program(n_cores=N_CORES, n_img=N_IMG):
    key = (n_cores, n_img)
    if key not in _CACHE:
        _CACHE[key] = build_program(n_cores, n_img)
    return _CACHE[key]


KERNEL_MODE = "two"  # "two" (stats launch + conv launch) or "fused"


def kernel(x, gamma, beta, W, b, _trace=False):
    if KERNEL_MODE == "two":
        return kernel_two(x, gamma, beta, W, b, _trace=_trace)
    x = np.ascontiguousarray(x, dtype=np.float32)
    assert x.shape[0] == N_CORES * N_IMG, x.shape
    nc = _get_program(N_CORES, N_IMG)
    in_maps = []
    for c in range(N_CORES):
        in_maps.append(
            {
                "x": x[c * N_IMG : (c + 1) * N_IMG],
                "gamma": np.ascontiguousarray(gamma, np.float32),
                "beta": np.ascontiguousarray(beta, np.float32),
                "W": np.ascontiguousarray(W, np.float32),
                "b": np.ascontiguousarray(b, np.float32),
            }
        )
    res = run_bass_kernel_spmd(
        nc, in_maps, core_ids=list(range(N_CORES)), trace=_trace
    )
    out = np.concatenate([res.results[c]["y"] for c in range(N_CORES)], axis=0)
    if _trace:
        kernel._last_result = res
    return out


# ====================== two-launch (collective-free) ======================

def build_stats_program(n_cores=N_CORES, n_img=N_IMG):
    """k1: per-core BN partial sums -> s_out [128, 2] = (sum x, sum x^2),
    partition p = 64*(n//2) + c over this core's images."""
    nhalf = n_img // 2
    nc = bacc.Bacc(
        "TRN2", target_bir_lowering=False, debug=False, num_devices=n_cores
    )
    x = nc.dram_tensor("x", [n_img, C, H, W], F32, kind="ExternalInput")
    s_out = nc.dram_tensor("s_out", [128, 2], F32, kind="ExternalOutput")

    with tile.TileContext(nc) as tc, ExitStack() as ctx:
        n_chunks = nhalf * NQ
        xchp = ctx.enter_context(tc.tile_pool(name="xch", bufs=n_chunks))
        statp = ctx.enter_context(tc.tile_pool(name="stat", bufs=1))
        sums = statp.tile([128, n_chunks], F32)
        sqs = statp.tile([128, n_chunks], F32)
        sqscr = statp.tile([128, QW], F32)
        xchs = []
        for n2 in range(nhalf):
            for q in range(NQ):
                xch = xchp.tile([128, QW], F32, tag="xch")
                xchs.append(xch)
                for half in range(2):
                    n = half * nhalf + n2
                    nc.sync.dma_start(
                        out=xch[half * C : half * C + C, :].rearrange(
                            "c (h w) -> c h w", w=W
                        ),
                        in_=x.ap()[n, :, q * Q_ROWS : (q + 1) * Q_ROWS, :],
                    )
        for idx, xch in enumerate(xchs):
            nc.vector.tensor_reduce(
                out=sums[:, idx : idx + 1], in_=xch,
                axis=mybir.AxisListType.X, op=mybir.AluOpType.add,
            )
            nc.scalar.activation(
                out=sqscr, in_=xch,
                func=mybir.ActivationFunctionType.Square,
                accum_out=sqs[:, idx : idx + 1],
            )
        res = statp.tile([128, 2], F32)
        nc.vector.tensor_reduce(
            out=res[:, 0:1], in_=sums,
            axis=mybir.AxisListType.X, op=mybir.AluOpType.add,
        )
        nc.vector.tensor_reduce(
            out=res[:, 1:2], in_=sqs,
            axis=mybir.AxisListType.X, op=mybir.AluOpType.add,
        )
        nc.sync.dma_start(out=s_out.ap(), in_=res)

    nc.compile()
    return nc


def build_conv_program(n_cores=N_CORES, n_img=N_IMG):
    """k2: binarize (thresholds given) + conv + relu, streaming x."""
    nhalf = n_img // 2
    nc = bacc.Bacc(
        "TRN2", target_bir_lowering=False, debug=False, num_devices=n_cores
    )
    x = nc.dram_tensor("x", [n_img, C, H, W], F32, kind="ExternalInput")
    Wt = nc.dram_tensor("W", [C, C, 3, 3], F32, kind="ExternalInput")
    bt = nc.dram_tensor("b", [C], F32, kind="ExternalInput")
    av = nc.dram_tensor("avec", [C], F32, kind="ExternalInput")
    dv = nc.dram_tensor("dvec", [C], F32, kind="ExternalInput")
    y = nc.dram_tensor("y", [n_img, C, H, W], F32, kind="ExternalOutput")

    with tile.TileContext(nc) as tc, ExitStack() as ctx:
        const = ctx.enter_context(tc.tile_pool(name="const", bufs=1))
        n_chunks = nhalf * NQ
        xchp = ctx.enter_context(tc.tile_pool(name="xch", bufs=n_chunks - 2))
        tmpp = ctx.enter_context(tc.tile_pool(name="tmpb", bufs=2))
        xbp = ctx.enter_context(tc.tile_pool(name="xb", bufs=4))
        psump = ctx.enter_context(tc.tile_pool(name="ps", bufs=4, space="PSUM"))
        pstr = ctx.enter_context(tc.tile_pool(name="pst", bufs=2, space="PSUM"))
        outp = ctx.enter_context(tc.tile_pool(name="out", bufs=3))

        identity64 = const.tile([C, C], F32)
        make_identity(nc, identity64)

        # all x chunk loads up front on the sync queue
        xchs = {}
        for n2 in range(nhalf):
            for q in range(NQ):
                xch = xchp.tile([128, QW], F32, tag="xch")
                xchs[(n2, q)] = xch
                for half in range(2):
                    n = half * nhalf + n2
                    nc.sync.dma_start(
                        out=xch[half * C : half * C + C, :].rearrange(
                            "c (h w) -> c h w", w=W
                        ),
                        in_=x.ap()[n, :, q * Q_ROWS : (q + 1) * Q_ROWS, :],
                    )

        # const DMAs on gpsimd
        wsb = const.tile([C, C, 9], F32)
        nc.gpsimd.dma_start(
            out=wsb, in_=Wt.ap().rearrange("o c kh kw -> o c (kh kw)")
        )
        b2 = const.tile([128, 1], F32)
        bsrc = bt.ap().rearrange("(c u) -> c u", u=1)
        nc.gpsimd.dma_start(out=b2[0:C, :], in_=bsrc)
        nc.gpsimd.dma_start(out=b2[C:128, :], in_=bsrc)
        a2 = const.tile([128, 1], F32)
        asrc = av.ap().rearrange("(c u) -> c u", u=1)
        nc.gpsimd.dma_start(out=a2[0:C, :], in_=asrc)
        nc.gpsimd.dma_start(out=a2[C:128, :], in_=asrc)
        d2 = const.tile([128, 1], F32)
        dsrc = dv.ap().rearrange("(c u) -> c u", u=1)
        nc.gpsimd.dma_start(out=d2[0:C, :], in_=dsrc)
        nc.gpsimd.dma_start(out=d2[C:128, :], in_=dsrc)

        # fp16 weight views via PE transposes
        w2 = const.tile([128, 9, C], WDT)
        for t in range(9):
            psT = pstr.tile([C, C], F32, tag="pst")
            nc.tensor.transpose(psT, wsb[:, :, t], identity64)
            nc.scalar.activation(
                out=w2[0:C, t, :], in_=psT,
                func=mybir.ActivationFunctionType.Copy,
            )
            if t >= 3:
                nc.scalar.activation(
                    out=w2[C:128, t - 3, :], in_=psT,
                    func=mybir.ActivationFunctionType.Copy,
                )

        out_engines = (nc.sync, nc.scalar)
        state = {"dma": 0}

        def conv_slot(n, xbv, s):
            h0 = s * 2 * ROWS_PER_CHUNK
            h1 = h0 + ROWS_PER_CHUNK
            P = psump.tile([128, NMM], F32, tag="psum")
            mms = []
            for kw in range(3):
                for cg, hb in ((0, h0), (64, h1)):
                    mms.append((cg, hb, kw, True))
            for kw in range(3):
                for cg, hb in ((0, h0), (64, h1)):
                    mms.append((cg, hb, kw, False))
            cg_seen = set()
            cg_last = {cg: max(i for i, m in enumerate(mms) if m[0] == cg)
                       for cg in (0, 64)}
            for i, (cg, hb, kw, is_pair) in enumerate(mms):
                if is_pair:
                    lhsT = w2[:, kw, :]
                    rhs = xbv[:, hb : hb + ROWS_PER_CHUNK, kw : kw + W]
                else:
                    lhsT = w2[0:C, 6 + kw, :]
                    rhs = xbv[0:C, hb + 2 : hb + 2 + ROWS_PER_CHUNK,
                              kw : kw + W]
                nc.tensor.matmul(
                    P[cg : cg + C, :], lhsT, rhs,
                    start=(cg not in cg_seen), stop=(i == cg_last[cg]),
                    tile_position=(0, cg), skip_group_check=True,
                )
                cg_seen.add(cg)
            osb = outp.tile([128, NMM], F32, tag="osb")
            if s % 2 == 0:
                nc.scalar.activation(
                    out=osb, in_=P,
                    func=mybir.ActivationFunctionType.Relu, bias=b2,
                )
            else:
                nc.vector.tensor_scalar(
                    out=osb, in0=P, scalar1=b2, scalar2=0.0,
                    op0=mybir.AluOpType.add, op1=mybir.AluOpType.max,
                )
            ov = osb.rearrange("p (h w) -> p h w", w=W)
            e0 = out_engines[state["dma"] % 2]
            e1 = out_engines[(state["dma"] + 1) % 2]
            state["dma"] += 2
            e0.dma_start(
                out=y.ap()[n, :, h0 : h0 + ROWS_PER_CHUNK, :],
                in_=ov[0:C, :, :],
            )
            e1.dma_start(
                out=y.ap()[n, :, h1 : h1 + ROWS_PER_CHUNK, :],
                in_=ov[C:128, :, :],
            )

        # stream: per image-pair, per chunk: fused Sign -> distribute ->
        # row-shifted copy -> conv slots as rows become available
        slot_hi = [(Q_ROWS * (q + 1) - 9) // 8 for q in range(NQ)]
        slot_hi[-1] = N_SLOTS - 1
        for n2 in range(nhalf):
            imgs = (n2, nhalf + n2)
            xbts, xbvs = [], []
            for n in imgs:
                xbt = xbp.tile([128, IMG], WDT, tag="xb")
                xbv = xbt.rearrange("p (hp wp) -> p hp wp", wp=WP)
                xbts.append(xbt)
                xbvs.append(xbv)
                nc.gpsimd.memset(xbv[0:C, 0:1, :], 0.0)
                nc.gpsimd.memset(xbv[0:C, HP - 1 : HP, :], 0.0)
                nc.gpsimd.memset(xbv[0:C, 1 : HP - 1, 0:1], 0.0)
                nc.gpsimd.memset(xbv[0:C, 1 : HP - 1, WP - 1 : WP], 0.0)
            slot_done = [0, 0]
            for q in range(NQ):
                xch = xchs[(n2, q)]
                h0c = q * Q_ROWS
                h1c = (q + 1) * Q_ROWS
                tmpb = tmpp.tile([128, QW], WDT, tag="tmpb")
                nc.scalar.activation(
                    out=tmpb, in_=xch,
                    func=mybir.ActivationFunctionType.Sign,
                    scale=a2, bias=d2,
                )
                for half in range(2):
                    nc.vector.tensor_copy(
                        out=xbvs[half][0:C, 1 + h0c : 1 + h1c, 1 : WP - 1],
                        in_=tmpb[half * C : half * C + C, :].rearrange(
                            "c (h w) -> c h w", w=W
                        ),
                    )
                    lo = 0 if q == 0 else (h0c - 1) * WP
                    hi_ = (h1c - 1) * WP if h1c < H else IMG - WP
                    nc.vector.tensor_copy(
                        out=xbts[half][C:128, lo:hi_],
                        in_=xbts[half][0:C, lo + WP : hi_ + WP],
                    )
                for half in range(2):
                    for s in range(slot_done[half], slot_hi[q] + 1):
                        conv_slot(imgs[half], xbvs[half], s)
                    slot_done[half] = slot_hi[q] + 1

    nc.compile()
    return nc


def _get_two_programs(n_cores=N_CORES, n_img=N_IMG):
    key = ("two", n_cores, n_img)
    if key not in _CACHE:
        _CACHE[key] = (
            build_stats_program(n_cores, n_img),
            build_conv_program(n_cores, n_img),
        )
    return _CACHE[key]


def kernel_two(x, gamma, beta, W, b, _trace=False):
    x = np.ascontiguousarray(x, dtype=np.float32)
    gamma = np.ascontiguousarray(gamma, np.float32)
    beta = np.ascontiguousarray(beta, np.float32)
    W = np.ascontiguousarray(W, np.float32)
    b = np.ascontiguousarray(b, np.float32)
    assert x.shape[0] == N_CORES * N_IMG, x.shape
    nc1, nc2 = _get_two_programs(N_CORES, N_IMG)
    shards = [x[c * N_IMG : (c + 1) * N_IMG] for c in range(N_CORES)]
    res1 = run_bass_kernel_spmd(
        nc1, [{"x": s} for s in shards],
        core_ids=list(range(N_CORES)), trace=_trace,
    )
    parts = np.stack([res1.results[c]["s_out"] for c in range(N_CORES)])
    tot = parts.astype(np.float64).sum(axis=0)
    tot64 = tot[:C] + tot[C:]
    count = float(N_CORES * N_IMG * PIX)
    mean = tot64[:, 0] / count
    var = tot64[:, 1] / count - mean * mean
    sigma = np.sqrt(var + EPS)
    avec = gamma.astype(np.float64)
    dvec = (beta.astype(np.float64) * sigma - avec * mean).astype(np.float32)
    avec = avec.astype(np.float32)
    res2 = run_bass_kernel_spmd(
        nc2,
        [{"x": s, "W": W, "b": b, "avec": avec, "dvec": dvec} for s in shards],
        core_ids=list(range(N_CORES)), trace=_trace,
    )
    out = np.concatenate([res2.results[c]["y"] for c in range(N_CORES)], axis=0)
    if _trace:
        kernel._last_result = (res1, res2)
    return out


# revision 45
# speedup vs baseline: 1.0753x; 1.0180x over previous
"""Trainium2 Bass kernel for BinConv2d:
   y = relu(conv2d(sign(batchnorm_train(x)), W, pad=1) + b)

Sharding: data-parallel over batch, 4 images per core on 8 cores.

Single SPMD launch: per-core BN partial sums (DVE reduce + ACT square) are
combined across cores with a tiny [128,2] AllReduce; sign() needs only a
per-channel affine threshold (sign(gamma*x + (beta*sigma - gamma*mean))),
so the variance path never touches per-element math.

Conv is 9 "taps" of a 64->64 matmul over all pixels. Binarized activations
(exact +-1 in fp16) are stored zero-padded [64ch, 114*114] per image, plus
a row-shifted duplicate on partitions 64..127 so taps (kh,kw) and (kh+1,kw)
pair into one K=128 matmul. Two 4-row output chunks run concurrently on
the two column halves of the PE array via tile_position.

The conv is rhs-stream-bound (~1 column/cycle into the PE); a short
full-array warm-up burst gated on the AllReduce result covers the HAM
clock ramp before the conv stream starts. A sacrificial AllReduce issued
at t~0 absorbs the inter-core rendezvous barrier and the ncfw wake-up
so the real AllReduce starts immediately once stats are ready.

Image order 0,2,1,3 keeps at most 3 binarized images resident (SBUF).
"""

import sys
from contextlib import ExitStack

import numpy as np

try:
    import concourse.bass as bass  # noqa: F401
except ImportError:  # pragma: no cover
    sys.path.insert(0, "/opt/trn_rl_repo")
    import concourse.bass as bass  # noqa: F401

import concourse.bacc as bacc
import concourse.tile as tile
from concourse import mybir
from concourse.bass_utils import run_bass_kernel_spmd
from concourse.masks import make_identity

F32 = mybir.dt.float32
WDT = mybir.dt.float16  # dtype for conv weights and binarized activations

N_CORES = 8
N_IMG = 4  # images per core (batch 32 / 8 cores)
NHALF = N_IMG // 2
C = 64
H = 112
W = 112
HP = H + 2  # 114
WP = W + 2  # 114
IMG = HP * WP  # 12996
EPS = 1e-4

PIX = H * W
Q_ROWS = 28  # rows per x load chunk
NQ = H // Q_ROWS  # 4
QW = Q_ROWS * W  # 3136
ROWS_PER_CHUNK = 4  # output rows per matmul chunk (N = 4*112 = 448)
NMM = ROWS_PER_CHUNK * W  # 448
N_SLOTS = H // (2 * ROWS_PER_CHUNK)  # 14

N_WARM_POST = 8  # warm-up burst gated on the AllReduce result


def build_program(n_cores=N_CORES, n_img=N_IMG):
    assert n_img % 2 == 0
    nhalf = n_img // 2
    fpart = nhalf * PIX

    nc = bacc.Bacc(
        "TRN2", target_bir_lowering=False, debug=False, num_devices=n_cores
    )
    x = nc.dram_tensor("x", [n_img, C, H, W], F32, kind="ExternalInput")
    gamma = nc.dram_tensor("gamma", [C], F32, kind="ExternalInput")
    beta = nc.dram_tensor("beta", [C], F32, kind="ExternalInput")
    Wt = nc.dram_tensor("W", [C, C, 3, 3], F32, kind="ExternalInput")
    bt = nc.dram_tensor("b", [C], F32, kind="ExternalInput")
    y = nc.dram_tensor("y", [n_img, C, H, W], F32, kind="ExternalOutput")

    with tile.TileContext(nc) as tc, ExitStack() as ctx:
        const = ctx.enter_context(tc.tile_pool(name="const", bufs=1))
        bigp = ctx.enter_context(tc.tile_pool(name="big", bufs=1))
        xbp = ctx.enter_context(tc.tile_pool(name="xb", bufs=3))
        statp = ctx.enter_context(tc.tile_pool(name="stat", bufs=1))
        psump = ctx.enter_context(tc.tile_pool(name="ps", bufs=3, space="PSUM"))
        psdum = ctx.enter_context(tc.tile_pool(name="psd", bufs=3, space="PSUM"))
        pstr = ctx.enter_context(tc.tile_pool(name="pst", bufs=2, space="PSUM"))
        outp = ctx.enter_context(tc.tile_pool(name="out", bufs=3))
        dramp = ctx.enter_context(tc.tile_pool(name="dram", bufs=1, space="DRAM"))

        # ---- constants / dummies ----
        wdum = const.tile([128, C], F32)
        nc.gpsimd.memset(wdum, 1.0)
        wdum2 = const.tile([128, C], F32)
        nc.gpsimd.memset(wdum2, 1.0)
        identity64 = const.tile([C, C], F32)
        make_identity(nc, identity64)
        eps64 = const.tile([C, 1], F32)
        nc.gpsimd.memset(eps64, EPS)

        xsb = bigp.tile([128, fpart], F32)
        xsb_v = xsb.rearrange("p (n2 h w) -> p n2 h w", n2=nhalf, h=H)

        dum_i = 0

        def dummy_mm(rhs_base, lhsT=None):
            nonlocal dum_i
            psD = psdum.tile([C, NMM], F32, tag="psd")
            nc.tensor.matmul(
                psD,
                wdum if lhsT is None else lhsT,
                xsb[:, rhs_base : rhs_base + NMM],
                start=True,
                stop=True,
                skip_group_check=True,
            )
            dum_i += 1

        # ---- all x load triggers up front (descriptor-cheap APs):
        # 16 DMAs of [64ch, 28*112 contiguous], alternating sync/scalar ----
        for n2 in range(nhalf):
            for q in range(NQ):
                base = n2 * PIX + q * QW
                for half in range(2):
                    n = half * nhalf + n2
                    dst = xsb[half * C : half * C + C, base : base + QW]
                    eng = nc.sync
                    eng.dma_start(
                        out=dst.rearrange("c (h w) -> c h w", w=W),
                        in_=x.ap()[n, :, q * Q_ROWS : (q + 1) * Q_ROWS, :],
                    )
        # const DMAs on gpsimd (won't block the load queues)
        wsb = const.tile([C, C, 9], F32)
        nc.gpsimd.dma_start(
            out=wsb, in_=Wt.ap().rearrange("o c kh kw -> o c (kh kw)")
        )
        b2 = const.tile([128, 1], F32)
        bsrc = bt.ap().rearrange("(c u) -> c u", u=1)
        nc.gpsimd.dma_start(out=b2[0:C, :], in_=bsrc)
        nc.gpsimd.dma_start(out=b2[C:128, :], in_=bsrc)
        gamma2 = const.tile([128, 1], F32)
        gsrc = gamma.ap().rearrange("(c u) -> c u", u=1)
        nc.gpsimd.dma_start(out=gamma2[0:C, :], in_=gsrc)
        nc.gpsimd.dma_start(out=gamma2[C:128, :], in_=gsrc)
        beta64 = const.tile([C, 1], F32)
        nc.gpsimd.dma_start(
            out=beta64, in_=beta.ap().rearrange("(c u) -> c u", u=1)
        )

        # ---- per-chunk BN partials (DVE sum, ACT sum-of-squares) ----
        n_chunks = nhalf * NQ
        sums = statp.tile([128, n_chunks], F32)
        sqs = statp.tile([128, n_chunks], F32)
        sqscr = statp.tile([128, QW], F32)
        for idx in range(n_chunks):
            base = idx * QW  # (n2, q) in row-major == contiguous slices
            nc.vector.tensor_reduce(
                out=sums[:, idx : idx + 1],
                in_=xsb[:, base : base + QW],
                axis=mybir.AxisListType.X,
                op=mybir.AluOpType.add,
            )
            nc.scalar.activation(
                out=sqscr,
                in_=xsb[:, base : base + QW],
                func=mybir.ActivationFunctionType.Square,
                accum_out=sqs[:, idx : idx + 1],
            )

        # fp16 weight views while stats run: w2[0:64,t,:] = tap t,
        # w2[64:128,t,:] = tap t+3 (PE transposes produce lhsT[c,o])
        w2 = const.tile([128, 9, C], WDT)
        for t in range(9):
            psT = pstr.tile([C, C], F32, tag="pst")
            nc.tensor.transpose(psT, wsb[:, :, t], identity64)
            nc.scalar.activation(
                out=w2[0:C, t, :], in_=psT,
                func=mybir.ActivationFunctionType.Copy,
            )
            if t >= 3:
                nc.scalar.activation(
                    out=w2[C:128, t - 3, :], in_=psT,
                    func=mybir.ActivationFunctionType.Copy,
                )

        # ---- AllReduce of (sum x, sum x^2) ----
        arin = statp.tile([128, 2], F32)
        nc.vector.tensor_reduce(
            out=arin[:, 0:1], in_=sums,
            axis=mybir.AxisListType.X, op=mybir.AluOpType.add,
        )
        nc.vector.tensor_reduce(
            out=arin[:, 1:2], in_=sqs,
            axis=mybir.AxisListType.X, op=mybir.AluOpType.add,
        )
        cc_in = dramp.tile([128, 2], F32)
        cc_out = dramp.tile([128, 2], F32)
        nc.sync.dma_start(out=cc_in, in_=arin)
        if n_cores > 1:
            nc.gpsimd.collective_compute(
                "AllReduce",
                mybir.AluOpType.add,
                replica_groups=[list(range(n_cores))],
                ins=[cc_in[:].opt()],
                outs=[cc_out[:].opt()],
            )
        else:
            nc.gpsimd.dma_start(out=cc_out, in_=cc_in)
        ar = statp.tile([128, 2], F32)
        nc.sync.dma_start(out=ar, in_=cc_out)
        # post-AR warm-up matmuls gate on wdum2, which depends on ar
        nc.vector.tensor_scalar_mul(wdum2[:, 0:2], ar, 0.0)

        # ---- fold -> per-channel threshold: d = beta*sigma - gamma*mean
        total_count = n_cores * 2 * fpart
        hi = statp.tile([C, 2], F32)
        nc.scalar.activation(
            out=hi, in_=ar[C:128, :], func=mybir.ActivationFunctionType.Copy
        )
        tot = statp.tile([C, 2], F32)
        nc.vector.tensor_add(out=tot, in0=ar[0:C, :], in1=hi)
        mean64 = statp.tile([C, 1], F32)
        nc.vector.tensor_scalar_mul(mean64, tot[:, 0:1], 1.0 / total_count)
        e2 = statp.tile([C, 1], F32)
        nc.vector.tensor_scalar_mul(e2, tot[:, 1:2], 1.0 / total_count)
        var64 = statp.tile([C, 1], F32)
        nc.vector.tensor_mul(out=var64, in0=mean64, in1=mean64)
        nc.vector.tensor_sub(out=var64, in0=e2, in1=var64)
        sigma = statp.tile([C, 1], F32)
        nc.scalar.activation(
            out=sigma, in_=var64,
            func=mybir.ActivationFunctionType.Sqrt, bias=eps64,
        )
        d64 = statp.tile([C, 1], F32)
        nc.vector.tensor_mul(out=d64, in0=beta64, in1=sigma)
        t2 = statp.tile([C, 1], F32)
        nc.vector.tensor_mul(out=t2, in0=gamma2[0:C, :], in1=mean64)
        nc.vector.tensor_sub(out=d64, in0=d64, in1=t2)
        d2 = statp.tile([128, 1], F32)
        nc.vector.tensor_copy(out=d2[0:C, :], in_=d64)
        nc.scalar.activation(
            out=d2[C:128, :], in_=d64, func=mybir.ActivationFunctionType.Copy
        )

        # PE warm-up burst while the threshold fold + first binarize run
        for i in range(N_WARM_POST):
            dummy_mm((i % (fpart // NMM)) * NMM, lhsT=wdum2)

        # ---- conv slot ----
        out_engines = (nc.sync, nc.scalar)
        out_dma_i = 0

        def conv_slot(n, xbv, s):
            nonlocal out_dma_i
            h0 = s * 2 * ROWS_PER_CHUNK
            h1 = h0 + ROWS_PER_CHUNK
            P = psump.tile([128, NMM], F32, tag="psum")
            mms = []
            for kw in range(3):
                for cg, hb in ((0, h0), (64, h1)):
                    mms.append((cg, hb, kw, True))
            for kw in range(3):
                for cg, hb in ((0, h0), (64, h1)):
                    mms.append((cg, hb, kw, False))
            cg_seen = set()
            cg_last = {cg: max(i for i, m in enumerate(mms) if m[0] == cg)
                       for cg in (0, 64)}
            for i, (cg, hb, kw, is_pair) in enumerate(mms):
                if is_pair:
                    lhsT = w2[:, kw, :]
                    rhs = xbv[:, hb : hb + ROWS_PER_CHUNK, kw : kw + W]
                else:
                    lhsT = w2[0:C, 6 + kw, :]
                    rhs = xbv[0:C, hb + 2 : hb + 2 + ROWS_PER_CHUNK, kw : kw + W]
                nc.tensor.matmul(
                    P[cg : cg + C, :],
                    lhsT,
                    rhs,
                    start=(cg not in cg_seen),
                    stop=(i == cg_last[cg]),
                    tile_position=(0, cg),
                    skip_group_check=True,
                )
                cg_seen.add(cg)
            # epilogue relu(P + b): alternate ACT/DVE
            osb = outp.tile([128, NMM], F32, tag="osb")
            if s % 2 == 0:
                nc.scalar.activation(
                    out=osb, in_=P,
                    func=mybir.ActivationFunctionType.Relu, bias=b2,
                )
            else:
                nc.vector.tensor_scalar(
                    out=osb, in0=P, scalar1=b2, scalar2=0.0,
                    op0=mybir.AluOpType.add, op1=mybir.AluOpType.max,
                )
            ov = osb.rearrange("p (h w) -> p h w", w=W)
            e0 = out_engines[out_dma_i % 2]
            e1 = out_engines[(out_dma_i + 1) % 2]
            out_dma_i += 2
            e0.dma_start(
                out=y.ap()[n, :, h0 : h0 + ROWS_PER_CHUNK, :],
                in_=ov[0:C, :, :],
            )
            e1.dma_start(
                out=y.ap()[n, :, h1 : h1 + ROWS_PER_CHUNK, :],
                in_=ov[C:128, :, :],
            )

        # ---- per image (order 0,2,1,3 to cap live xb tiles at 3):
        # binarize in 2 row-chunks then conv ----
        h_split = 56
        img_order = []
        for n2 in range(nhalf):
            img_order += [n2, nhalf + n2]
        for n in img_order:
            half = n // nhalf
            n2 = n % nhalf
            xbt = xbp.tile([128, IMG], WDT, tag="xb")
            xbv = xbt.rearrange("p (hp wp) -> p hp wp", wp=WP)
            nc.gpsimd.memset(xbv[0:C, 0:1, :], 0.0)
            nc.gpsimd.memset(xbv[0:C, HP - 1 : HP, :], 0.0)
            nc.gpsimd.memset(xbv[0:C, 1 : HP - 1, 0:1], 0.0)
            nc.gpsimd.memset(xbv[0:C, 1 : HP - 1, WP - 1 : WP], 0.0)
            for ci, (h0c, h1c) in enumerate(((0, h_split), (h_split, H))):
                nc.scalar.activation(
                    out=xbv[0:C, 1 + h0c : 1 + h1c, 1 : WP - 1],
                    in_=xsb_v[half * C : half * C + C, n2, h0c:h1c, :],
                    func=mybir.ActivationFunctionType.Sign,
                    scale=gamma2[half * C : half * C + C, :],
                    bias=d2[half * C : half * C + C, :],
                )
                # copy B rows = A rows + 1 (chunked, no holes; last chunk
                # runs through row 112 -- A row 113 is the zero border)
                lo = 0 if ci == 0 else (h0c - 1) * WP
                hi_ = (h1c - 1) * WP if h1c < H else IMG - WP
                nc.vector.tensor_copy(
                    out=xbt[C:128, lo:hi_], in_=xbt[0:C, lo + WP : hi_ + WP]
                )
                s_lo = 0 if ci == 0 else (h_split - 9) // 8 + 1
                s_hi = (h1c - 9) // 8 if h1c < H else N_SLOTS - 1
                for s in range(s_lo, s_hi + 1):
                    conv_slot(n, xbv, s)

    nc.compile()
    return nc


_CACHE = {}


def _get_program(n_cores=N_CORES, n_img=N_IMG):
    key = (n_cores, n_img)
    if key not in _CACHE:
        _CACHE[key] = build_program(n_cores, n_img)
    return _CACHE[key]


KERNEL_MODE = "two"  # "two" (stats launch + conv launch) or "fused"


def kernel(x, gamma, beta, W, b, _trace=False):
    if KERNEL_MODE == "two":
        return kernel_two(x, gamma, beta, W, b, _trace=_trace)
    x = np.ascontiguousarray(x, dtype=np.float32)
    assert x.shape[0] == N_CORES * N_IMG, x.shape
    nc = _get_program(N_CORES, N_IMG)
    in_maps = []
    for c in range(N_CORES):
        in_maps.append(
            {
                "x": x[c * N_IMG : (c + 1) * N_IMG],
                "gamma": np.ascontiguousarray(gamma, np.float32),
                "beta": np.ascontiguousarray(beta, np.float32),
                "W": np.ascontiguousarray(W, np.float32),
                "b": np.ascontiguousarray(b, np.float32),
            }
        )
    res = run_bass_kernel_spmd(
        nc, in_maps, core_ids=list(range(N_CORES)), trace=_trace
    )
    out = np.concatenate([res.results[c]["y"] for c in range(N_CORES)], axis=0)
    if _trace:
        kernel._last_result = res
    return out


# ====================== two-launch (collective-free) ======================

def build_stats_program(n_cores=N_CORES, n_img=N_IMG):
    """k1: per-core BN partial sums -> s_out [128, 2] = (sum x, sum x^2),
    partition p = 64*(n//2) + c over this core's images."""
    nhalf = n_img // 2
    nc = bacc.Bacc(
        "TRN2", target_bir_lowering=False, debug=False, num_devices=n_cores
    )
    x = nc.dram_tensor("x", [n_img, C, H, W], F32, kind="ExternalInput")
    s_out = nc.dram_tensor("s_out", [128, 2], F32, kind="ExternalOutput")

    with tile.TileContext(nc) as tc, ExitStack() as ctx:
        n_chunks = nhalf * NQ
        xchp = ctx.enter_context(tc.tile_pool(name="xch", bufs=n_chunks))
        statp = ctx.enter_context(tc.tile_pool(name="stat", bufs=1))
        sums = statp.tile([128, n_chunks], F32)
        sqs = statp.tile([128, n_chunks], F32)
        sqscr = statp.tile([128, QW], F32)
        xchs = []
        for n2 in range(nhalf):
            for q in range(NQ):
                xch = xchp.tile([128, QW], F32, tag="xch")
                xchs.append(xch)
                for half in range(2):
                    n = half * nhalf + n2
                    nc.sync.dma_start(
                        out=xch[half * C : half * C + C, :].rearrange(
                            "c (h w) -> c h w", w=W
                        ),
                        in_=x.ap()[n, :, q * Q_ROWS : (q + 1) * Q_ROWS, :],
                    )
        for idx, xch in enumerate(xchs):
            nc.vector.tensor_reduce(
                out=sums[:, idx : idx + 1], in_=xch,
                axis=mybir.AxisListType.X, op=mybir.AluOpType.add,
            )
            nc.scalar.activation(
                out=sqscr, in_=xch,
                func=mybir.ActivationFunctionType.Square,
                accum_out=sqs[:, idx : idx + 1],
            )
        res = statp.tile([128, 2], F32)
        nc.vector.tensor_reduce(
            out=res[:, 0:1], in_=sums,
            axis=mybir.AxisListType.X, op=mybir.AluOpType.add,
        )
        nc.vector.tensor_reduce(
            out=res[:, 1:2], in_=sqs,
            axis=mybir.AxisListType.X, op=mybir.AluOpType.add,
        )
        nc.sync.dma_start(out=s_out.ap(), in_=res)

    nc.compile()
    return nc


def build_conv_program(n_cores=N_CORES, n_img=N_IMG):
    """k2: binarize (thresholds given) + conv + relu, streaming x."""
    nhalf = n_img // 2
    nc = bacc.Bacc(
        "TRN2", target_bir_lowering=False, debug=False, num_devices=n_cores
    )
    x = nc.dram_tensor("x", [n_img, C, H, W], F32, kind="ExternalInput")
    Wt = nc.dram_tensor("W", [C, C, 3, 3], F32, kind="ExternalInput")
    bt = nc.dram_tensor("b", [C], F32, kind="ExternalInput")
    av = nc.dram_tensor("avec", [C], F32, kind="ExternalInput")
    dv = nc.dram_tensor("dvec", [C], F32, kind="ExternalInput")
    y = nc.dram_tensor("y", [n_img, C, H, W], F32, kind="ExternalOutput")

    with tile.TileContext(nc) as tc, ExitStack() as ctx:
        const = ctx.enter_context(tc.tile_pool(name="const", bufs=1))
        n_chunks = nhalf * NQ
        xchp = ctx.enter_context(tc.tile_pool(name="xch", bufs=n_chunks - 3))
        tmpp = ctx.enter_context(tc.tile_pool(name="tmpb", bufs=2))
        xbp = ctx.enter_context(tc.tile_pool(name="xb", bufs=4))
        psump = ctx.enter_context(tc.tile_pool(name="ps", bufs=6, space="PSUM"))
        pstr = ctx.enter_context(tc.tile_pool(name="pst", bufs=2, space="PSUM"))
        outp = ctx.enter_context(tc.tile_pool(name="out", bufs=8))

        identity64 = const.tile([C, C], F32)
        make_identity(nc, identity64)

        # all x chunk loads up front on the sync queue
        xchs = {}
        for n2 in range(nhalf):
            for q in range(NQ):
                xch = xchp.tile([128, QW], F32, tag="xch")
                xchs[(n2, q)] = xch
                for half in range(2):
                    n = half * nhalf + n2
                    nc.sync.dma_start(
                        out=xch[half * C : half * C + C, :].rearrange(
                            "c (h w) -> c h w", w=W
                        ),
                        in_=x.ap()[n, :, q * Q_ROWS : (q + 1) * Q_ROWS, :],
                    )

        # const DMAs on gpsimd
        wsb = const.tile([C, C, 9], F32)
        nc.gpsimd.dma_start(
            out=wsb, in_=Wt.ap().rearrange("o c kh kw -> o c (kh kw)")
        )
        b2 = const.tile([128, 1], F32)
        bsrc = bt.ap().rearrange("(c u) -> c u", u=1)
        nc.gpsimd.dma_start(out=b2[0:C, :], in_=bsrc)
        nc.gpsimd.dma_start(out=b2[C:128, :], in_=bsrc)
        a2 = const.tile([128, 1], F32)
        asrc = av.ap().rearrange("(c u) -> c u", u=1)
        nc.gpsimd.dma_start(out=a2[0:C, :], in_=asrc)
        nc.gpsimd.dma_start(out=a2[C:128, :], in_=asrc)
        d2 = const.tile([128, 1], F32)
        dsrc = dv.ap().rearrange("(c u) -> c u", u=1)
        nc.gpsimd.dma_start(out=d2[0:C, :], in_=dsrc)
        nc.gpsimd.dma_start(out=d2[C:128, :], in_=dsrc)

        # fp16 weight views via PE transposes
        w2 = const.tile([128, 9, C], WDT)
        for t in range(9):
            psT = pstr.tile([C, C], F32, tag="pst")
            nc.tensor.transpose(psT, wsb[:, :, t], identity64)
            nc.scalar.activation(
                out=w2[0:C, t, :], in_=psT,
                func=mybir.ActivationFunctionType.Copy,
            )
            if t >= 3:
                nc.scalar.activation(
                    out=w2[C:128, t - 3, :], in_=psT,
                    func=mybir.ActivationFunctionType.Copy,
                )

        out_engines = (nc.sync, nc.scalar)
        state = {"dma": 0}

        def conv_slot(n, xbv, s):
            h0 = s * 2 * ROWS_PER_CHUNK
            h1 = h0 + ROWS_PER_CHUNK
            P = psump.tile([128, NMM], F32, tag="psum")
            mms = []
            for kw in range(3):
                for cg, hb in ((0, h0), (64, h1)):
                    mms.append((cg, hb, kw, True))
            for kw in range(3):
                for cg, hb in ((0, h0), (64, h1)):
                    mms.append((cg, hb, kw, False))
            cg_seen = set()
            cg_last = {cg: max(i for i, m in enumerate(mms) if m[0] == cg)
                       for cg in (0, 64)}
            for i, (cg, hb, kw, is_pair) in enumerate(mms):
                if is_pair:
                    lhsT = w2[:, kw, :]
                    rhs = xbv[:, hb : hb + ROWS_PER_CHUNK, kw : kw + W]
                else:
                    lhsT = w2[0:C, 6 + kw, :]
                    rhs = xbv[0:C, hb + 2 : hb + 2 + ROWS_PER_CHUNK,
                              kw : kw + W]
                nc.tensor.matmul(
                    P[cg : cg + C, :], lhsT, rhs,
                    start=(cg not in cg_seen), stop=(i == cg_last[cg]),
                    tile_position=(0, cg), skip_group_check=True,
                )
                cg_seen.add(cg)
            osb = outp.tile([128, NMM], F32, tag="osb")
            if s % 2 == 0:
                nc.scalar.activation(
                    out=osb, in_=P,
                    func=mybir.ActivationFunctionType.Relu, bias=b2,
                )
            else:
                nc.vector.tensor_scalar(
                    out=osb, in0=P, scalar1=b2, scalar2=0.0,
                    op0=mybir.AluOpType.add, op1=mybir.AluOpType.max,
                )
            ov = osb.rearrange("p (h w) -> p h w", w=W)
            e0 = out_engines[state["dma"] % 2]
            e1 = out_engines[(state["dma"] + 1) % 2]
            state["dma"] += 2
            e0.dma_start(
                out=y.ap()[n, :, h0 : h0 + ROWS_PER_CHUNK, :],
                in_=ov[0:C, :, :],
            )
            e1.dma_start(
                out=y.ap()[n, :, h1 : h1 + ROWS_PER_CHUNK, :],
                in_=ov[C:128, :, :],
            )

        # stream: per image-pair, per chunk: fused Sign -> distribute ->
        # row-shifted copy -> conv slots as rows become available
        slot_hi = [(Q_ROWS * (q + 1) - 9) // 8 for q in range(NQ)]
        slot_hi[-1] = N_SLOTS - 1
        for n2 in range(nhalf):
            imgs = (n2, nhalf + n2)
            xbts, xbvs = [], []
            for n in imgs:
                xbt = xbp.tile([128, IMG], WDT, tag="xb")
                xbv = xbt.rearrange("p (hp wp) -> p hp wp", wp=WP)
                xbts.append(xbt)
                xbvs.append(xbv)
                nc.gpsimd.memset(xbv[0:C, 0:1, :], 0.0)
                nc.gpsimd.memset(xbv[0:C, HP - 1 : HP, :], 0.0)
                nc.gpsimd.memset(xbv[0:C, 1 : HP - 1, 0:1], 0.0)
                nc.gpsimd.memset(xbv[0:C, 1 : HP - 1, WP - 1 : WP], 0.0)
            slot_done = [0, 0]
            for q in range(NQ):
                xch = xchs[(n2, q)]
                h0c = q * Q_ROWS
                h1c = (q + 1) * Q_ROWS
                tmpb = tmpp.tile([128, QW], WDT, tag="tmpb")
                nc.scalar.activation(
                    out=tmpb, in_=xch,
                    func=mybir.ActivationFunctionType.Sign,
                    scale=a2, bias=d2,
                )
                for half in range(2):
                    nc.vector.tensor_copy(
                        out=xbvs[half][0:C, 1 + h0c : 1 + h1c, 1 : WP - 1],
                        in_=tmpb[half * C : half * C + C, :].rearrange(
                            "c (h w) -> c h w", w=W
                        ),
                    )
                    lo = 0 if q == 0 else (h0c - 1) * WP
                    hi_ = (h1c - 1) * WP if h1c < H else IMG - WP
                    nc.vector.tensor_copy(
                        out=xbts[half][C:128, lo:hi_],
                        in_=xbts[half][0:C, lo + WP : hi_ + WP],
                    )
                for half in range(2):
                    for s in range(slot_done[half], slot_hi[q] + 1):
                        conv_slot(imgs[half], xbvs[half], s)
                    slot_done[half] = slot_hi[q] + 1

    nc.compile()
    return nc


def _get_two_programs(n_cores=N_CORES, n_img=N_IMG):
    key = ("two", n_cores, n_img)
    if key not in _CACHE:
        _CACHE[key] = (
            build_stats_program(n_cores, n_img),
            build_conv_program(n_cores, n_img),
        )
    return _CACHE[key]


def kernel_two(x, gamma, beta, W, b, _trace=False):
    x = np.ascontiguousarray(x, dtype=np.float32)
    gamma = np.ascontiguousarray(gamma, np.float32)
    beta = np.ascontiguousarray(beta, np.float32)
    W = np.ascontiguousarray(W, np.float32)
    b = np.ascontiguousarray(b, np.float32)
    assert x.shape[0] == N_CORES * N_IMG, x.shape
    nc1, nc2 = _get_two_programs(N_CORES, N_IMG)
    shards = [x[c * N_IMG : (c + 1) * N_IMG] for c in range(N_CORES)]
    res1 = run_bass_kernel_spmd(
        nc1, [{"x": s} for s in shards],
        core_ids=list(range(N_CORES)), trace=_trace,
    )
    parts = np.stack([res1.results[c]["s_out"] for c in range(N_CORES)])
    tot = parts.astype(np.float64).sum(axis=0)
    tot64 = tot[:C] + tot[C:]
    count = float(N_CORES * N_IMG * PIX)
    mean = tot64[:, 0] / count
    var = tot64[:, 1] / count - mean * mean
    sigma = np.sqrt(var + EPS)
    avec = gamma.astype(np.float64)
    dvec = (beta.astype(np.float64) * sigma - avec * mean).astype(np.float32)
    avec = avec.astype(np.float32)
    res2 = run_bass_kernel_spmd(
        nc2,
        [{"x": s, "W": W, "b": b, "avec": avec, "dvec": dvec} for s in shards],
        core_ids=list(range(N_CORES)), trace=_trace,
    )
    out = np.concatenate([res2.results[c]["y"] for c in range(N_CORES)], axis=0)
    if _trace:
        kernel._last_result = (res1, res2)
    return out


# revision 46
# speedup vs baseline: 1.0789x; 1.0033x over previous
"""Trainium2 Bass kernel for BinConv2d:
   y = relu(conv2d(sign(batchnorm_train(x)), W, pad=1) + b)

Sharding: data-parallel over batch, 4 images per core on 8 cores.

Single SPMD launch: per-core BN partial sums (DVE reduce + ACT square) are
combined across cores with a tiny [128,2] AllReduce; sign() needs only a
per-channel affine threshold (sign(gamma*x + (beta*sigma - gamma*mean))),
so the variance path never touches per-element math.

Conv is 9 "taps" of a 64->64 matmul over all pixels. Binarized activations
(exact +-1 in fp16) are stored zero-padded [64ch, 114*114] per image, plus
a row-shifted duplicate on partitions 64..127 so taps (kh,kw) and (kh+1,kw)
pair into one K=128 matmul. Two 4-row output chunks run concurrently on
the two column halves of the PE array via tile_position.

The conv is rhs-stream-bound (~1 column/cycle into the PE); a short
full-array warm-up burst gated on the AllReduce result covers the HAM
clock ramp before the conv stream starts. A sacrificial AllReduce issued
at t~0 absorbs the inter-core rendezvous barrier and the ncfw wake-up
so the real AllReduce starts immediately once stats are ready.

Image order 0,2,1,3 keeps at most 3 binarized images resident (SBUF).
"""

import sys
from contextlib import ExitStack

import numpy as np

try:
    import concourse.bass as bass  # noqa: F401
except ImportError:  # pragma: no cover
    sys.path.insert(0, "/opt/trn_rl_repo")
    import concourse.bass as bass  # noqa: F401

import concourse.bacc as bacc
import concourse.tile as tile
from concourse import mybir
from concourse.bass_utils import run_bass_kernel_spmd
from concourse.masks import make_identity

F32 = mybir.dt.float32
WDT = mybir.dt.float16  # dtype for conv weights and binarized activations

N_CORES = 8
N_IMG = 4  # images per core (batch 32 / 8 cores)
NHALF = N_IMG // 2
C = 64
H = 112
W = 112
HP = H + 2  # 114
WP = W + 2  # 114
IMG = HP * WP  # 12996
EPS = 1e-4

PIX = H * W
Q_ROWS = 28  # rows per x load chunk
NQ = H // Q_ROWS  # 4
QW = Q_ROWS * W  # 3136
ROWS_PER_CHUNK = 4  # output rows per matmul chunk (N = 4*112 = 448)
NMM = ROWS_PER_CHUNK * W  # 448
N_SLOTS = H // (2 * ROWS_PER_CHUNK)  # 14

N_WARM_POST = 8  # warm-up burst gated on the AllReduce result


def build_program(n_cores=N_CORES, n_img=N_IMG):
    assert n_img % 2 == 0
    nhalf = n_img // 2
    fpart = nhalf * PIX

    nc = bacc.Bacc(
        "TRN2", target_bir_lowering=False, debug=False, num_devices=n_cores
    )
    x = nc.dram_tensor("x", [n_img, C, H, W], F32, kind="ExternalInput")
    gamma = nc.dram_tensor("gamma", [C], F32, kind="ExternalInput")
    beta = nc.dram_tensor("beta", [C], F32, kind="ExternalInput")
    Wt = nc.dram_tensor("W", [C, C, 3, 3], F32, kind="ExternalInput")
    bt = nc.dram_tensor("b", [C], F32, kind="ExternalInput")
    y = nc.dram_tensor("y", [n_img, C, H, W], F32, kind="ExternalOutput")

    with tile.TileContext(nc) as tc, ExitStack() as ctx:
        const = ctx.enter_context(tc.tile_pool(name="const", bufs=1))
        bigp = ctx.enter_context(tc.tile_pool(name="big", bufs=1))
        xbp = ctx.enter_context(tc.tile_pool(name="xb", bufs=3))
        statp = ctx.enter_context(tc.tile_pool(name="stat", bufs=1))
        psump = ctx.enter_context(tc.tile_pool(name="ps", bufs=3, space="PSUM"))
        psdum = ctx.enter_context(tc.tile_pool(name="psd", bufs=3, space="PSUM"))
        pstr = ctx.enter_context(tc.tile_pool(name="pst", bufs=2, space="PSUM"))
        outp = ctx.enter_context(tc.tile_pool(name="out", bufs=3))
        dramp = ctx.enter_context(tc.tile_pool(name="dram", bufs=1, space="DRAM"))

        # ---- constants / dummies ----
        wdum = const.tile([128, C], F32)
        nc.gpsimd.memset(wdum, 1.0)
        wdum2 = const.tile([128, C], F32)
        nc.gpsimd.memset(wdum2, 1.0)
        identity64 = const.tile([C, C], F32)
        make_identity(nc, identity64)
        eps64 = const.tile([C, 1], F32)
        nc.gpsimd.memset(eps64, EPS)

        xsb = bigp.tile([128, fpart], F32)
        xsb_v = xsb.rearrange("p (n2 h w) -> p n2 h w", n2=nhalf, h=H)

        dum_i = 0

        def dummy_mm(rhs_base, lhsT=None):
            nonlocal dum_i
            psD = psdum.tile([C, NMM], F32, tag="psd")
            nc.tensor.matmul(
                psD,
                wdum if lhsT is None else lhsT,
                xsb[:, rhs_base : rhs_base + NMM],
                start=True,
                stop=True,
                skip_group_check=True,
            )
            dum_i += 1

        # ---- all x load triggers up front (descriptor-cheap APs):
        # 16 DMAs of [64ch, 28*112 contiguous], alternating sync/scalar ----
        for n2 in range(nhalf):
            for q in range(NQ):
                base = n2 * PIX + q * QW
                for half in range(2):
                    n = half * nhalf + n2
                    dst = xsb[half * C : half * C + C, base : base + QW]
                    eng = nc.sync
                    eng.dma_start(
                        out=dst.rearrange("c (h w) -> c h w", w=W),
                        in_=x.ap()[n, :, q * Q_ROWS : (q + 1) * Q_ROWS, :],
                    )
        # const DMAs on gpsimd (won't block the load queues)
        wsb = const.tile([C, C, 9], F32)
        nc.gpsimd.dma_start(
            out=wsb, in_=Wt.ap().rearrange("o c kh kw -> o c (kh kw)")
        )
        b2 = const.tile([128, 1], F32)
        bsrc = bt.ap().rearrange("(c u) -> c u", u=1)
        nc.gpsimd.dma_start(out=b2[0:C, :], in_=bsrc)
        nc.gpsimd.dma_start(out=b2[C:128, :], in_=bsrc)
        gamma2 = const.tile([128, 1], F32)
        gsrc = gamma.ap().rearrange("(c u) -> c u", u=1)
        nc.gpsimd.dma_start(out=gamma2[0:C, :], in_=gsrc)
        nc.gpsimd.dma_start(out=gamma2[C:128, :], in_=gsrc)
        beta64 = const.tile([C, 1], F32)
        nc.gpsimd.dma_start(
            out=beta64, in_=beta.ap().rearrange("(c u) -> c u", u=1)
        )

        # ---- per-chunk BN partials (DVE sum, ACT sum-of-squares) ----
        n_chunks = nhalf * NQ
        sums = statp.tile([128, n_chunks], F32)
        sqs = statp.tile([128, n_chunks], F32)
        sqscr = statp.tile([128, QW], F32)
        for idx in range(n_chunks):
            base = idx * QW  # (n2, q) in row-major == contiguous slices
            nc.vector.tensor_reduce(
                out=sums[:, idx : idx + 1],
                in_=xsb[:, base : base + QW],
                axis=mybir.AxisListType.X,
                op=mybir.AluOpType.add,
            )
            nc.scalar.activation(
                out=sqscr,
                in_=xsb[:, base : base + QW],
                func=mybir.ActivationFunctionType.Square,
                accum_out=sqs[:, idx : idx + 1],
            )

        # fp16 weight views while stats run: w2[0:64,t,:] = tap t,
        # w2[64:128,t,:] = tap t+3 (PE transposes produce lhsT[c,o])
        w2 = const.tile([128, 9, C], WDT)
        for t in range(9):
            psT = pstr.tile([C, C], F32, tag="pst")
            nc.tensor.transpose(psT, wsb[:, :, t], identity64)
            nc.scalar.activation(
                out=w2[0:C, t, :], in_=psT,
                func=mybir.ActivationFunctionType.Copy,
            )
            if t >= 3:
                nc.scalar.activation(
                    out=w2[C:128, t - 3, :], in_=psT,
                    func=mybir.ActivationFunctionType.Copy,
                )

        # ---- AllReduce of (sum x, sum x^2) ----
        arin = statp.tile([128, 2], F32)
        nc.vector.tensor_reduce(
            out=arin[:, 0:1], in_=sums,
            axis=mybir.AxisListType.X, op=mybir.AluOpType.add,
        )
        nc.vector.tensor_reduce(
            out=arin[:, 1:2], in_=sqs,
            axis=mybir.AxisListType.X, op=mybir.AluOpType.add,
        )
        cc_in = dramp.tile([128, 2], F32)
        cc_out = dramp.tile([128, 2], F32)
        nc.sync.dma_start(out=cc_in, in_=arin)
        if n_cores > 1:
            nc.gpsimd.collective_compute(
                "AllReduce",
                mybir.AluOpType.add,
                replica_groups=[list(range(n_cores))],
                ins=[cc_in[:].opt()],
                outs=[cc_out[:].opt()],
            )
        else:
            nc.gpsimd.dma_start(out=cc_out, in_=cc_in)
        ar = statp.tile([128, 2], F32)
        nc.sync.dma_start(out=ar, in_=cc_out)
        # post-AR warm-up matmuls gate on wdum2, which depends on ar
        nc.vector.tensor_scalar_mul(wdum2[:, 0:2], ar, 0.0)

        # ---- fold -> per-channel threshold: d = beta*sigma - gamma*mean
        total_count = n_cores * 2 * fpart
        hi = statp.tile([C, 2], F32)
        nc.scalar.activation(
            out=hi, in_=ar[C:128, :], func=mybir.ActivationFunctionType.Copy
        )
        tot = statp.tile([C, 2], F32)
        nc.vector.tensor_add(out=tot, in0=ar[0:C, :], in1=hi)
        mean64 = statp.tile([C, 1], F32)
        nc.vector.tensor_scalar_mul(mean64, tot[:, 0:1], 1.0 / total_count)
        e2 = statp.tile([C, 1], F32)
        nc.vector.tensor_scalar_mul(e2, tot[:, 1:2], 1.0 / total_count)
        var64 = statp.tile([C, 1], F32)
        nc.vector.tensor_mul(out=var64, in0=mean64, in1=mean64)
        nc.vector.tensor_sub(out=var64, in0=e2, in1=var64)
        sigma = statp.tile([C, 1], F32)
        nc.scalar.activation(
            out=sigma, in_=var64,
            func=mybir.ActivationFunctionType.Sqrt, bias=eps64,
        )
        d64 = statp.tile([C, 1], F32)
        nc.vector.tensor_mul(out=d64, in0=beta64, in1=sigma)
        t2 = statp.tile([C, 1], F32)
        nc.vector.tensor_mul(out=t2, in0=gamma2[0:C, :], in1=mean64)
        nc.vector.tensor_sub(out=d64, in0=d64, in1=t2)
        d2 = statp.tile([128, 1], F32)
        nc.vector.tensor_copy(out=d2[0:C, :], in_=d64)
        nc.scalar.activation(
            out=d2[C:128, :], in_=d64, func=mybir.ActivationFunctionType.Copy
        )

        # PE warm-up burst while the threshold fold + first binarize run
        for i in range(N_WARM_POST):
            dummy_mm((i % (fpart // NMM)) * NMM, lhsT=wdum2)

        # ---- conv slot ----
        out_engines = (nc.sync, nc.scalar)
        out_dma_i = 0

        def conv_slot(n, xbv, s):
            nonlocal out_dma_i
            h0 = s * 2 * ROWS_PER_CHUNK
            h1 = h0 + ROWS_PER_CHUNK
            P = psump.tile([128, NMM], F32, tag="psum")
            mms = []
            for kw in range(3):
                for cg, hb in ((0, h0), (64, h1)):
                    mms.append((cg, hb, kw, True))
            for kw in range(3):
                for cg, hb in ((0, h0), (64, h1)):
                    mms.append((cg, hb, kw, False))
            cg_seen = set()
            cg_last = {cg: max(i for i, m in enumerate(mms) if m[0] == cg)
                       for cg in (0, 64)}
            for i, (cg, hb, kw, is_pair) in enumerate(mms):
                if is_pair:
                    lhsT = w2[:, kw, :]
                    rhs = xbv[:, hb : hb + ROWS_PER_CHUNK, kw : kw + W]
                else:
                    lhsT = w2[0:C, 6 + kw, :]
                    rhs = xbv[0:C, hb + 2 : hb + 2 + ROWS_PER_CHUNK, kw : kw + W]
                nc.tensor.matmul(
                    P[cg : cg + C, :],
                    lhsT,
                    rhs,
                    start=(cg not in cg_seen),
                    stop=(i == cg_last[cg]),
                    tile_position=(0, cg),
                    skip_group_check=True,
                )
                cg_seen.add(cg)
            # epilogue relu(P + b): alternate ACT/DVE
            osb = outp.tile([128, NMM], F32, tag="osb")
            if s % 2 == 0:
                nc.scalar.activation(
                    out=osb, in_=P,
                    func=mybir.ActivationFunctionType.Relu, bias=b2,
                )
            else:
                nc.vector.tensor_scalar(
                    out=osb, in0=P, scalar1=b2, scalar2=0.0,
                    op0=mybir.AluOpType.add, op1=mybir.AluOpType.max,
                )
            ov = osb.rearrange("p (h w) -> p h w", w=W)
            e0 = out_engines[out_dma_i % 2]
            e1 = out_engines[(out_dma_i + 1) % 2]
            out_dma_i += 2
            e0.dma_start(
                out=y.ap()[n, :, h0 : h0 + ROWS_PER_CHUNK, :],
                in_=ov[0:C, :, :],
            )
            e1.dma_start(
                out=y.ap()[n, :, h1 : h1 + ROWS_PER_CHUNK, :],
                in_=ov[C:128, :, :],
            )

        # ---- per image (order 0,2,1,3 to cap live xb tiles at 3):
        # binarize in 2 row-chunks then conv ----
        h_split = 56
        img_order = []
        for n2 in range(nhalf):
            img_order += [n2, nhalf + n2]
        for n in img_order:
            half = n // nhalf
            n2 = n % nhalf
            xbt = xbp.tile([128, IMG], WDT, tag="xb")
            xbv = xbt.rearrange("p (hp wp) -> p hp wp", wp=WP)
            nc.gpsimd.memset(xbv[0:C, 0:1, :], 0.0)
            nc.gpsimd.memset(xbv[0:C, HP - 1 : HP, :], 0.0)
            nc.gpsimd.memset(xbv[0:C, 1 : HP - 1, 0:1], 0.0)
            nc.gpsimd.memset(xbv[0:C, 1 : HP - 1, WP - 1 : WP], 0.0)
            for ci, (h0c, h1c) in enumerate(((0, h_split), (h_split, H))):
                nc.scalar.activation(
                    out=xbv[0:C, 1 + h0c : 1 + h1c, 1 : WP - 1],
                    in_=xsb_v[half * C : half * C + C, n2, h0c:h1c, :],
                    func=mybir.ActivationFunctionType.Sign,
                    scale=gamma2[half * C : half * C + C, :],
                    bias=d2[half * C : half * C + C, :],
                )
                # copy B rows = A rows + 1 (chunked, no holes; last chunk
                # runs through row 112 -- A row 113 is the zero border)
                lo = 0 if ci == 0 else (h0c - 1) * WP
                hi_ = (h1c - 1) * WP if h1c < H else IMG - WP
                nc.vector.tensor_copy(
                    out=xbt[C:128, lo:hi_], in_=xbt[0:C, lo + WP : hi_ + WP]
                )
                s_lo = 0 if ci == 0 else (h_split - 9) // 8 + 1
                s_hi = (h1c - 9) // 8 if h1c < H else N_SLOTS - 1
                for s in range(s_lo, s_hi + 1):
                    conv_slot(n, xbv, s)

    nc.compile()
    return nc


_CACHE = {}


def _get_program(n_cores=N_CORES, n_img=N_IMG):
    key = (n_cores, n_img)
    if key not in _CACHE:
        _CACHE[key] = build_program(n_cores, n_img)
    return _CACHE[key]


KERNEL_MODE = "fused"  # "two" (stats launch + conv launch) or "fused"


def kernel(x, gamma, beta, W, b, _trace=False):
    if KERNEL_MODE == "two":
        return kernel_two(x, gamma, beta, W, b, _trace=_trace)
    x = np.ascontiguousarray(x, dtype=np.float32)
    assert x.shape[0] == N_CORES * N_IMG, x.shape
    nc = _get_program(N_CORES, N_IMG)
    in_maps = []
    for c in range(N_CORES):
        in_maps.append(
            {
                "x": x[c * N_IMG : (c + 1) * N_IMG],
                "gamma": np.ascontiguousarray(gamma, np.float32),
                "beta": np.ascontiguousarray(beta, np.float32),
                "W": np.ascontiguousarray(W, np.float32),
                "b": np.ascontiguousarray(b, np.float32),
            }
        )
    res = run_bass_kernel_spmd(
        nc, in_maps, core_ids=list(range(N_CORES)), trace=_trace
    )
    out = np.concatenate([res.results[c]["y"] for c in range(N_CORES)], axis=0)
    if _trace:
        kernel._last_result = res
    return out


# ====================== two-launch (collective-free) ======================

def build_stats_program(n_cores=N_CORES, n_img=N_IMG):
    """k1: per-core BN partial sums -> s_out [128, 2] = (sum x, sum x^2),
    partition p = 64*(n//2) + c over this core's images."""
    nhalf = n_img // 2
    nc = bacc.Bacc(
        "TRN2", target_bir_lowering=False, debug=False, num_devices=n_cores
    )
    x = nc.dram_tensor("x", [n_img, C, H, W], F32, kind="ExternalInput")
    s_out = nc.dram_tensor("s_out", [128, 2], F32, kind="ExternalOutput")

    with tile.TileContext(nc) as tc, ExitStack() as ctx:
        n_chunks = nhalf * NQ
        xchp = ctx.enter_context(tc.tile_pool(name="xch", bufs=n_chunks))
        statp = ctx.enter_context(tc.tile_pool(name="stat", bufs=1))
        sums = statp.tile([128, n_chunks], F32)
        sqs = statp.tile([128, n_chunks], F32)
        sqscr = statp.tile([128, QW], F32)
        xchs = []
        for n2 in range(nhalf):
            for q in range(NQ):
                xch = xchp.tile([128, QW], F32, tag="xch")
                xchs.append(xch)
                for half in range(2):
                    n = half * nhalf + n2
                    nc.sync.dma_start(
                        out=xch[half * C : half * C + C, :].rearrange(
                            "c (h w) -> c h w", w=W
                        ),
                        in_=x.ap()[n, :, q * Q_ROWS : (q + 1) * Q_ROWS, :],
                    )
        for idx, xch in enumerate(xchs):
            nc.vector.tensor_reduce(
                out=sums[:, idx : idx + 1], in_=xch,
                axis=mybir.AxisListType.X, op=mybir.AluOpType.add,
            )
            nc.scalar.activation(
                out=sqscr, in_=xch,
                func=mybir.ActivationFunctionType.Square,
                accum_out=sqs[:, idx : idx + 1],
            )
        res = statp.tile([128, 2], F32)
        nc.vector.tensor_reduce(
            out=res[:, 0:1], in_=sums,
            axis=mybir.AxisListType.X, op=mybir.AluOpType.add,
        )
        nc.vector.tensor_reduce(
            out=res[:, 1:2], in_=sqs,
            axis=mybir.AxisListType.X, op=mybir.AluOpType.add,
        )
        nc.sync.dma_start(out=s_out.ap(), in_=res)

    nc.compile()
    return nc


def build_conv_program(n_cores=N_CORES, n_img=N_IMG):
    """k2: binarize (thresholds given) + conv + relu, streaming x."""
    nhalf = n_img // 2
    nc = bacc.Bacc(
        "TRN2", target_bir_lowering=False, debug=False, num_devices=n_cores
    )
    x = nc.dram_tensor("x", [n_img, C, H, W], F32, kind="ExternalInput")
    Wt = nc.dram_tensor("W", [C, C, 3, 3], F32, kind="ExternalInput")
    bt = nc.dram_tensor("b", [C], F32, kind="ExternalInput")
    av = nc.dram_tensor("avec", [C], F32, kind="ExternalInput")
    dv = nc.dram_tensor("dvec", [C], F32, kind="ExternalInput")
    y = nc.dram_tensor("y", [n_img, C, H, W], F32, kind="ExternalOutput")

    with tile.TileContext(nc) as tc, ExitStack() as ctx:
        const = ctx.enter_context(tc.tile_pool(name="const", bufs=1))
        n_chunks = nhalf * NQ
        xchp = ctx.enter_context(tc.tile_pool(name="xch", bufs=n_chunks - 3))
        tmpp = ctx.enter_context(tc.tile_pool(name="tmpb", bufs=2))
        xbp = ctx.enter_context(tc.tile_pool(name="xb", bufs=4))
        psump = ctx.enter_context(tc.tile_pool(name="ps", bufs=6, space="PSUM"))
        pstr = ctx.enter_context(tc.tile_pool(name="pst", bufs=2, space="PSUM"))
        outp = ctx.enter_context(tc.tile_pool(name="out", bufs=8))

        identity64 = const.tile([C, C], F32)
        make_identity(nc, identity64)

        # all x chunk loads up front on the sync queue
        xchs = {}
        for n2 in range(nhalf):
            for q in range(NQ):
                xch = xchp.tile([128, QW], F32, tag="xch")
                xchs[(n2, q)] = xch
                for half in range(2):
                    n = half * nhalf + n2
                    nc.sync.dma_start(
                        out=xch[half * C : half * C + C, :].rearrange(
                            "c (h w) -> c h w", w=W
                        ),
                        in_=x.ap()[n, :, q * Q_ROWS : (q + 1) * Q_ROWS, :],
                    )

        # const DMAs on gpsimd
        wsb = const.tile([C, C, 9], F32)
        nc.gpsimd.dma_start(
            out=wsb, in_=Wt.ap().rearrange("o c kh kw -> o c (kh kw)")
        )
        b2 = const.tile([128, 1], F32)
        bsrc = bt.ap().rearrange("(c u) -> c u", u=1)
        nc.gpsimd.dma_start(out=b2[0:C, :], in_=bsrc)
        nc.gpsimd.dma_start(out=b2[C:128, :], in_=bsrc)
        a2 = const.tile([128, 1], F32)
        asrc = av.ap().rearrange("(c u) -> c u", u=1)
        nc.gpsimd.dma_start(out=a2[0:C, :], in_=asrc)
        nc.gpsimd.dma_start(out=a2[C:128, :], in_=asrc)
        d2 = const.tile([128, 1], F32)
        dsrc = dv.ap().rearrange("(c u) -> c u", u=1)
        nc.gpsimd.dma_start(out=d2[0:C, :], in_=dsrc)
        nc.gpsimd.dma_start(out=d2[C:128, :], in_=dsrc)

        # fp16 weight views via PE transposes
        w2 = const.tile([128, 9, C], WDT)
        for t in range(9):
            psT = pstr.tile([C, C], F32, tag="pst")
            nc.tensor.transpose(psT, wsb[:, :, t], identity64)
            nc.scalar.activation(
                out=w2[0:C, t, :], in_=psT,
                func=mybir.ActivationFunctionType.Copy,
            )
            if t >= 3:
                nc.scalar.activation(
                    out=w2[C:128, t - 3, :], in_=psT,
                    func=mybir.ActivationFunctionType.Copy,
                )

        out_engines = (nc.sync, nc.scalar)
        state = {"dma": 0}

        def conv_slot(n, xbv, s):
            h0 = s * 2 * ROWS_PER_CHUNK
            h1 = h0 + ROWS_PER_CHUNK
            P = psump.tile([128, NMM], F32, tag="psum")
            mms = []
            for kw in range(3):
                for cg, hb in ((0, h0), (64, h1)):
                    mms.append((cg, hb, kw, True))
            for kw in range(3):
                for cg, hb in ((0, h0), (64, h1)):
                    mms.append((cg, hb, kw, False))
            cg_seen = set()
            cg_last = {cg: max(i for i, m in enumerate(mms) if m[0] == cg)
                       for cg in (0, 64)}
            for i, (cg, hb, kw, is_pair) in enumerate(mms):
                if is_pair:
                    lhsT = w2[:, kw, :]
                    rhs = xbv[:, hb : hb + ROWS_PER_CHUNK, kw : kw + W]
                else:
                    lhsT = w2[0:C, 6 + kw, :]
                    rhs = xbv[0:C, hb + 2 : hb + 2 + ROWS_PER_CHUNK,
                              kw : kw + W]
                nc.tensor.matmul(
                    P[cg : cg + C, :], lhsT, rhs,
                    start=(cg not in cg_seen), stop=(i == cg_last[cg]),
                    tile_position=(0, cg), skip_group_check=True,
                )
                cg_seen.add(cg)
            osb = outp.tile([128, NMM], F32, tag="osb")
            if s % 2 == 0:
                nc.scalar.activation(
                    out=osb, in_=P,
                    func=mybir.ActivationFunctionType.Relu, bias=b2,
                )
            else:
                nc.vector.tensor_scalar(
                    out=osb, in0=P, scalar1=b2, scalar2=0.0,
                    op0=mybir.AluOpType.add, op1=mybir.AluOpType.max,
                )
            ov = osb.rearrange("p (h w) -> p h w", w=W)
            e0 = out_engines[state["dma"] % 2]
            e1 = out_engines[(state["dma"] + 1) % 2]
            state["dma"] += 2
            e0.dma_start(
                out=y.ap()[n, :, h0 : h0 + ROWS_PER_CHUNK, :],
                in_=ov[0:C, :, :],
            )
            e1.dma_start(
                out=y.ap()[n, :, h1 : h1 + ROWS_PER_CHUNK, :],
                in_=ov[C:128, :, :],
            )

        # stream: per image-pair, per chunk: fused Sign -> distribute ->
        # row-shifted copy -> conv slots as rows become available
        slot_hi = [(Q_ROWS * (q + 1) - 9) // 8 for q in range(NQ)]
        slot_hi[-1] = N_SLOTS - 1
        for n2 in range(nhalf):
            imgs = (n2, nhalf + n2)
            xbts, xbvs = [], []
            for n in imgs:
                xbt = xbp.tile([128, IMG], WDT, tag="xb")
                xbv = xbt.rearrange("p (hp wp) -> p hp wp", wp=WP)
                xbts.append(xbt)
                xbvs.append(xbv)
                nc.gpsimd.memset(xbv[0:C, 0:1, :], 0.0)
                nc.gpsimd.memset(xbv[0:C, HP - 1 : HP, :], 0.0)
                nc.gpsimd.memset(xbv[0:C, 1 : HP - 1, 0:1], 0.0)
                nc.gpsimd.memset(xbv[0:C, 1 : HP - 1, WP - 1 : WP], 0.0)
            slot_done = [0, 0]
            for q in range(NQ):
                xch = xchs[(n2, q)]
                h0c = q * Q_ROWS
                h1c = (q + 1) * Q_ROWS
                tmpb = tmpp.tile([128, QW], WDT, tag="tmpb")
                nc.scalar.activation(
                    out=tmpb, in_=xch,
                    func=mybir.ActivationFunctionType.Sign,
                    scale=a2, bias=d2,
                )
                for half in range(2):
                    nc.vector.tensor_copy(
                        out=xbvs[half][0:C, 1 + h0c : 1 + h1c, 1 : WP - 1],
                        in_=tmpb[half * C : half * C + C, :].rearrange(
                            "c (h w) -> c h w", w=W
                        ),
                    )
                    lo = 0 if q == 0 else (h0c - 1) * WP
                    hi_ = (h1c - 1) * WP if h1c < H else IMG - WP
                    nc.vector.tensor_copy(
                        out=xbts[half][C:128, lo:hi_],
                        in_=xbts[half][0:C, lo + WP : hi_ + WP],
                    )
                for half in range(2):
                    for s in range(slot_done[half], slot_hi[q] + 1):
                        conv_slot(imgs[half], xbvs[half], s)
                    slot_done[half] = slot_hi[q] + 1

    nc.compile()
    return nc


def _get_two_programs(n_cores=N_CORES, n_img=N_IMG):
    key = ("two", n_cores, n_img)
    if key not in _CACHE:
        _CACHE[key] = (
            build_stats_program(n_cores, n_img),
            build_conv_program(n_cores, n_img),
        )
    return _CACHE[key]


def kernel_two(x, gamma, beta, W, b, _trace=False):
    x = np.ascontiguousarray(x, dtype=np.float32)
    gamma = np.ascontiguousarray(gamma, np.float32)
    beta = np.ascontiguousarray(beta, np.float32)
    W = np.ascontiguousarray(W, np.float32)
    b = np.ascontiguousarray(b, np.float32)
    assert x.shape[0] == N_CORES * N_IMG, x.shape
    nc1, nc2 = _get_two_programs(N_CORES, N_IMG)
    shards = [x[c * N_IMG : (c + 1) * N_IMG] for c in range(N_CORES)]
    res1 = run_bass_kernel_spmd(
        nc1, [{"x": s} for s in shards],
        core_ids=list(range(N_CORES)), trace=_trace,
    )
    parts = np.stack([res1.results[c]["s_out"] for c in range(N_CORES)])
    tot = parts.astype(np.float64).sum(axis=0)
    tot64 = tot[:C] + tot[C:]
    count = float(N_CORES * N_IMG * PIX)
    mean = tot64[:, 0] / count
    var = tot64[:, 1] / count - mean * mean
    sigma = np.sqrt(var + EPS)
    avec = gamma.astype(np.float64)
    dvec = (beta.astype(np.float64) * sigma - avec * mean).astype(np.float32)
    avec = avec.astype(np.float32)
    res2 = run_bass_kernel_spmd(
        nc2,
        [{"x": s, "W": W, "b": b, "avec": avec, "dvec": dvec} for s in shards],
        core_ids=list(range(N_CORES)), trace=_trace,
    )
    out = np.concatenate([res2.results[c]["y"] for c in range(N_CORES)], axis=0)
    if _trace:
        kernel._last_result = (res1, res2)
    return out


# revision 47
# speedup vs baseline: 1.1711x; 1.0855x over previous
"""Trainium2 Bass kernel for BinConv2d:
   y = relu(conv2d(sign(batchnorm_train(x)), W, pad=1) + b)

Sharding: data-parallel over batch, 4 images per core on 8 cores.

Single SPMD launch: per-core BN partial sums (DVE reduce + ACT square) are
combined across cores with a tiny [128,2] AllReduce; sign() needs only a
per-channel affine threshold (sign(gamma*x + (beta*sigma - gamma*mean))),
so the variance path never touches per-element math.

Conv is 9 "taps" of a 64->64 matmul over all pixels. Binarized activations
(exact +-1 in fp16) are stored zero-padded [64ch, 114*114] per image, plus
a row-shifted duplicate on partitions 64..127 so taps (kh,kw) and (kh+1,kw)
pair into one K=128 matmul. Two 4-row output chunks run concurrently on
the two column halves of the PE array via tile_position.

The conv is rhs-stream-bound (~1 column/cycle into the PE); a short
full-array warm-up burst gated on the AllReduce result covers the HAM
clock ramp before the conv stream starts. A sacrificial AllReduce issued
at t~0 absorbs the inter-core rendezvous barrier and the ncfw wake-up
so the real AllReduce starts immediately once stats are ready.

Image order 0,2,1,3 keeps at most 3 binarized images resident (SBUF).
"""

import sys
from contextlib import ExitStack

import numpy as np

try:
    import concourse.bass as bass  # noqa: F401
except ImportError:  # pragma: no cover
    sys.path.insert(0, "/opt/trn_rl_repo")
    import concourse.bass as bass  # noqa: F401

import concourse.bacc as bacc
import concourse.tile as tile
from concourse import mybir
from concourse.bass_utils import run_bass_kernel_spmd
from concourse.masks import make_identity

F32 = mybir.dt.float32
WDT = mybir.dt.float16  # dtype for conv weights and binarized activations

N_CORES = 8
N_IMG = 4  # images per core (batch 32 / 8 cores)
NHALF = N_IMG // 2
C = 64
H = 112
W = 112
HP = H + 2  # 114
WP = W + 2  # 114
IMG = HP * WP  # 12996
EPS = 1e-4

PIX = H * W
Q_ROWS = 28  # rows per x load chunk
NQ = H // Q_ROWS  # 4
QW = Q_ROWS * W  # 3136
ROWS_PER_CHUNK = 4  # output rows per matmul chunk (N = 4*112 = 448)
NMM = ROWS_PER_CHUNK * W  # 448
N_SLOTS = H // (2 * ROWS_PER_CHUNK)  # 14

N_WARM_POST = 8  # warm-up burst gated on the AllReduce result


def build_program(n_cores=N_CORES, n_img=N_IMG):
    assert n_img % 2 == 0
    nhalf = n_img // 2
    fpart = nhalf * PIX

    nc = bacc.Bacc(
        "TRN2", target_bir_lowering=False, debug=False, num_devices=n_cores
    )
    x = nc.dram_tensor("x", [n_img, C, H, W], F32, kind="ExternalInput")
    gamma = nc.dram_tensor("gamma", [C], F32, kind="ExternalInput")
    beta = nc.dram_tensor("beta", [C], F32, kind="ExternalInput")
    Wt = nc.dram_tensor("W", [C, C, 3, 3], F32, kind="ExternalInput")
    bt = nc.dram_tensor("b", [C], F32, kind="ExternalInput")
    y = nc.dram_tensor("y", [n_img, C, H, W], F32, kind="ExternalOutput")

    with tile.TileContext(nc) as tc, ExitStack() as ctx:
        const = ctx.enter_context(tc.tile_pool(name="const", bufs=1))
        bigp = ctx.enter_context(tc.tile_pool(name="big", bufs=1))
        xbp = ctx.enter_context(tc.tile_pool(name="xb", bufs=3))
        statp = ctx.enter_context(tc.tile_pool(name="stat", bufs=1))
        psump = ctx.enter_context(tc.tile_pool(name="ps", bufs=3, space="PSUM"))
        psdum = ctx.enter_context(tc.tile_pool(name="psd", bufs=3, space="PSUM"))
        pstr = ctx.enter_context(tc.tile_pool(name="pst", bufs=2, space="PSUM"))
        outp = ctx.enter_context(tc.tile_pool(name="out", bufs=3))
        dramp = ctx.enter_context(tc.tile_pool(name="dram", bufs=1, space="DRAM"))

        # ---- constants / dummies ----
        wdum = const.tile([128, C], F32)
        nc.gpsimd.memset(wdum, 1.0)
        wdum2 = const.tile([128, C], F32)
        nc.gpsimd.memset(wdum2, 1.0)
        identity64 = const.tile([C, C], F32)
        make_identity(nc, identity64)
        eps64 = const.tile([C, 1], F32)
        nc.gpsimd.memset(eps64, EPS)

        xsb = bigp.tile([128, fpart], F32)
        xsb_v = xsb.rearrange("p (n2 h w) -> p n2 h w", n2=nhalf, h=H)

        dum_i = 0

        def dummy_mm(rhs_base, lhsT=None):
            nonlocal dum_i
            psD = psdum.tile([C, NMM], F32, tag="psd")
            nc.tensor.matmul(
                psD,
                wdum if lhsT is None else lhsT,
                xsb[:, rhs_base : rhs_base + NMM],
                start=True,
                stop=True,
                skip_group_check=True,
            )
            dum_i += 1

        # ---- all x load triggers up front (descriptor-cheap APs):
        # 16 DMAs of [64ch, 28*112 contiguous], alternating sync/scalar ----
        for n2 in range(nhalf):
            for q in range(NQ):
                base = n2 * PIX + q * QW
                for half in range(2):
                    n = half * nhalf + n2
                    dst = xsb[half * C : half * C + C, base : base + QW]
                    eng = nc.sync
                    eng.dma_start(
                        out=dst.rearrange("c (h w) -> c h w", w=W),
                        in_=x.ap()[n, :, q * Q_ROWS : (q + 1) * Q_ROWS, :],
                    )
        # const DMAs on gpsimd (won't block the load queues)
        wsb = const.tile([C, C, 9], F32)
        nc.gpsimd.dma_start(
            out=wsb, in_=Wt.ap().rearrange("o c kh kw -> o c (kh kw)")
        )
        b2 = const.tile([128, 1], F32)
        bsrc = bt.ap().rearrange("(c u) -> c u", u=1)
        nc.gpsimd.dma_start(out=b2[0:C, :], in_=bsrc)
        nc.gpsimd.dma_start(out=b2[C:128, :], in_=bsrc)
        gamma2 = const.tile([128, 1], F32)
        gsrc = gamma.ap().rearrange("(c u) -> c u", u=1)
        nc.gpsimd.dma_start(out=gamma2[0:C, :], in_=gsrc)
        nc.gpsimd.dma_start(out=gamma2[C:128, :], in_=gsrc)
        beta64 = const.tile([C, 1], F32)
        nc.gpsimd.dma_start(
            out=beta64, in_=beta.ap().rearrange("(c u) -> c u", u=1)
        )

        # ---- per-chunk BN partials (DVE sum, ACT sum-of-squares) ----
        n_chunks = nhalf * NQ
        sums = statp.tile([128, n_chunks], F32)
        sqs = statp.tile([128, n_chunks], F32)
        sqscr = statp.tile([128, QW], F32)
        for idx in range(n_chunks):
            base = idx * QW  # (n2, q) in row-major == contiguous slices
            nc.vector.tensor_reduce(
                out=sums[:, idx : idx + 1],
                in_=xsb[:, base : base + QW],
                axis=mybir.AxisListType.X,
                op=mybir.AluOpType.add,
            )
            nc.scalar.activation(
                out=sqscr,
                in_=xsb[:, base : base + QW],
                func=mybir.ActivationFunctionType.Square,
                accum_out=sqs[:, idx : idx + 1],
            )

        # fp16 weight views while stats run: w2[0:64,t,:] = tap t,
        # w2[64:128,t,:] = tap t+3 (PE transposes produce lhsT[c,o])
        w2 = const.tile([128, 9, C], WDT)
        for t in range(9):
            psT = pstr.tile([C, C], F32, tag="pst")
            nc.tensor.transpose(psT, wsb[:, :, t], identity64)
            nc.scalar.activation(
                out=w2[0:C, t, :], in_=psT,
                func=mybir.ActivationFunctionType.Copy,
            )
            if t >= 3:
                nc.scalar.activation(
                    out=w2[C:128, t - 3, :], in_=psT,
                    func=mybir.ActivationFunctionType.Copy,
                )

        # ---- AllReduce of (sum x, sum x^2) ----
        arin = statp.tile([128, 2], F32)
        nc.vector.tensor_reduce(
            out=arin[:, 0:1], in_=sums,
            axis=mybir.AxisListType.X, op=mybir.AluOpType.add,
        )
        nc.vector.tensor_reduce(
            out=arin[:, 1:2], in_=sqs,
            axis=mybir.AxisListType.X, op=mybir.AluOpType.add,
        )
        cc_in = dramp.tile([128, 2], F32)
        cc_out = dramp.tile([128, 2], F32)
        nc.sync.dma_start(out=cc_in, in_=arin)
        if n_cores > 1:
            nc.gpsimd.collective_compute(
                "AllReduce",
                mybir.AluOpType.add,
                replica_groups=[list(range(n_cores))],
                ins=[cc_in[:].opt()],
                outs=[cc_out[:].opt()],
            )
        else:
            nc.gpsimd.dma_start(out=cc_out, in_=cc_in)
        ar = statp.tile([128, 2], F32)
        nc.sync.dma_start(out=ar, in_=cc_out)
        # post-AR warm-up matmuls gate on wdum2, which depends on ar
        nc.vector.tensor_scalar_mul(wdum2[:, 0:2], ar, 0.0)

        # ---- fold -> per-channel threshold: d = beta*sigma - gamma*mean
        total_count = n_cores * 2 * fpart
        hi = statp.tile([C, 2], F32)
        nc.scalar.activation(
            out=hi, in_=ar[C:128, :], func=mybir.ActivationFunctionType.Copy
        )
        tot = statp.tile([C, 2], F32)
        nc.vector.tensor_add(out=tot, in0=ar[0:C, :], in1=hi)
        mean64 = statp.tile([C, 1], F32)
        nc.vector.tensor_scalar_mul(mean64, tot[:, 0:1], 1.0 / total_count)
        e2 = statp.tile([C, 1], F32)
        nc.vector.tensor_scalar_mul(e2, tot[:, 1:2], 1.0 / total_count)
        var64 = statp.tile([C, 1], F32)
        nc.vector.tensor_mul(out=var64, in0=mean64, in1=mean64)
        nc.vector.tensor_sub(out=var64, in0=e2, in1=var64)
        sigma = statp.tile([C, 1], F32)
        nc.scalar.activation(
            out=sigma, in_=var64,
            func=mybir.ActivationFunctionType.Sqrt, bias=eps64,
        )
        d64 = statp.tile([C, 1], F32)
        nc.vector.tensor_mul(out=d64, in0=beta64, in1=sigma)
        t2 = statp.tile([C, 1], F32)
        nc.vector.tensor_mul(out=t2, in0=gamma2[0:C, :], in1=mean64)
        nc.vector.tensor_sub(out=d64, in0=d64, in1=t2)
        d2 = statp.tile([128, 1], F32)
        nc.vector.tensor_copy(out=d2[0:C, :], in_=d64)
        nc.scalar.activation(
            out=d2[C:128, :], in_=d64, func=mybir.ActivationFunctionType.Copy
        )

        # PE warm-up burst while the threshold fold + first binarize run
        for i in range(N_WARM_POST):
            dummy_mm((i % (fpart // NMM)) * NMM, lhsT=wdum2)

        # ---- conv slot ----
        out_engines = (nc.sync, nc.scalar)
        out_dma_i = 0

        def conv_slot(n, xbv, s):
            nonlocal out_dma_i
            h0 = s * 2 * ROWS_PER_CHUNK
            h1 = h0 + ROWS_PER_CHUNK
            P = psump.tile([128, NMM], F32, tag="psum")
            mms = []
            for kw in range(3):
                for cg, hb in ((0, h0), (64, h1)):
                    mms.append((cg, hb, kw, True))
            for kw in range(3):
                for cg, hb in ((0, h0), (64, h1)):
                    mms.append((cg, hb, kw, False))
            cg_seen = set()
            cg_last = {cg: max(i for i, m in enumerate(mms) if m[0] == cg)
                       for cg in (0, 64)}
            for i, (cg, hb, kw, is_pair) in enumerate(mms):
                if is_pair:
                    lhsT = w2[:, kw, :]
                    rhs = xbv[:, hb : hb + ROWS_PER_CHUNK, kw : kw + W]
                else:
                    lhsT = w2[0:C, 6 + kw, :]
                    rhs = xbv[0:C, hb + 2 : hb + 2 + ROWS_PER_CHUNK, kw : kw + W]
                nc.tensor.matmul(
                    P[cg : cg + C, :],
                    lhsT,
                    rhs,
                    start=(cg not in cg_seen),
                    stop=(i == cg_last[cg]),
                    tile_position=(0, cg),
                    skip_group_check=True,
                )
                cg_seen.add(cg)
            # epilogue relu(P + b): alternate ACT/DVE
            osb = outp.tile([128, NMM], F32, tag="osb")
            if s % 2 == 0:
                nc.scalar.activation(
                    out=osb, in_=P,
                    func=mybir.ActivationFunctionType.Relu, bias=b2,
                )
            else:
                nc.vector.tensor_scalar(
                    out=osb, in0=P, scalar1=b2, scalar2=0.0,
                    op0=mybir.AluOpType.add, op1=mybir.AluOpType.max,
                )
            ov = osb.rearrange("p (h w) -> p h w", w=W)
            e0 = out_engines[out_dma_i % 2]
            e1 = out_engines[(out_dma_i + 1) % 2]
            out_dma_i += 2
            e0.dma_start(
                out=y.ap()[n, :, h0 : h0 + ROWS_PER_CHUNK, :],
                in_=ov[0:C, :, :],
            )
            e1.dma_start(
                out=y.ap()[n, :, h1 : h1 + ROWS_PER_CHUNK, :],
                in_=ov[C:128, :, :],
            )

        # ---- per image (order 0,2,1,3 to cap live xb tiles at 3):
        # binarize in 2 row-chunks then conv ----
        h_split = 56
        img_order = []
        for n2 in range(nhalf):
            img_order += [n2, nhalf + n2]
        for n in img_order:
            half = n // nhalf
            n2 = n % nhalf
            xbt = xbp.tile([128, IMG], WDT, tag="xb")
            xbv = xbt.rearrange("p (hp wp) -> p hp wp", wp=WP)
            nc.gpsimd.memset(xbv[0:C, 0:1, :], 0.0)
            nc.gpsimd.memset(xbv[0:C, HP - 1 : HP, :], 0.0)
            nc.gpsimd.memset(xbv[0:C, 1 : HP - 1, 0:1], 0.0)
            nc.gpsimd.memset(xbv[0:C, 1 : HP - 1, WP - 1 : WP], 0.0)
            for ci, (h0c, h1c) in enumerate(((0, h_split), (h_split, H))):
                nc.scalar.activation(
                    out=xbv[0:C, 1 + h0c : 1 + h1c, 1 : WP - 1],
                    in_=xsb_v[half * C : half * C + C, n2, h0c:h1c, :],
                    func=mybir.ActivationFunctionType.Sign,
                    scale=gamma2[half * C : half * C + C, :],
                    bias=d2[half * C : half * C + C, :],
                )
                # copy B rows = A rows + 1 (chunked, no holes; last chunk
                # runs through row 112 -- A row 113 is the zero border)
                lo = 0 if ci == 0 else (h0c - 1) * WP
                hi_ = (h1c - 1) * WP if h1c < H else IMG - WP
                nc.vector.tensor_copy(
                    out=xbt[C:128, lo:hi_], in_=xbt[0:C, lo + WP : hi_ + WP]
                )
                s_lo = 0 if ci == 0 else (h_split - 9) // 8 + 1
                s_hi = (h1c - 9) // 8 if h1c < H else N_SLOTS - 1
                for s in range(s_lo, s_hi + 1):
                    conv_slot(n, xbv, s)

    nc.compile()
    return nc


_CACHE = {}


def _get_program(n_cores=N_CORES, n_img=N_IMG):
    key = (n_cores, n_img)
    if key not in _CACHE:
        _CACHE[key] = build_program(n_cores, n_img)
    return _CACHE[key]


KERNEL_MODE = "two"  # "two" (stats launch + conv launch) or "fused"


def kernel(x, gamma, beta, W, b, _trace=False):
    if KERNEL_MODE == "two":
        return kernel_two(x, gamma, beta, W, b, _trace=_trace)
    x = np.ascontiguousarray(x, dtype=np.float32)
    assert x.shape[0] == N_CORES * N_IMG, x.shape
    nc = _get_program(N_CORES, N_IMG)
    in_maps = []
    for c in range(N_CORES):
        in_maps.append(
            {
                "x": x[c * N_IMG : (c + 1) * N_IMG],
                "gamma": np.ascontiguousarray(gamma, np.float32),
                "beta": np.ascontiguousarray(beta, np.float32),
                "W": np.ascontiguousarray(W, np.float32),
                "b": np.ascontiguousarray(b, np.float32),
            }
        )
    res = run_bass_kernel_spmd(
        nc, in_maps, core_ids=list(range(N_CORES)), trace=_trace
    )
    out = np.concatenate([res.results[c]["y"] for c in range(N_CORES)], axis=0)
    if _trace:
        kernel._last_result = res
    return out


# ====================== two-launch (collective-free) ======================

def build_stats_program(n_cores=N_CORES, n_img=N_IMG):
    """k1: per-core BN partial sums -> s_out [128, 2] = (sum x, sum x^2),
    partition p = 64*(n//2) + c over this core's images."""
    nhalf = n_img // 2
    nc = bacc.Bacc(
        "TRN2", target_bir_lowering=False, debug=False, num_devices=n_cores
    )
    x = nc.dram_tensor("x", [n_img, C, H, W], F32, kind="ExternalInput")
    s_out = nc.dram_tensor("s_out", [128, 2], F32, kind="ExternalOutput")

    with tile.TileContext(nc) as tc, ExitStack() as ctx:
        n_chunks = nhalf * NQ
        xchp = ctx.enter_context(tc.tile_pool(name="xch", bufs=n_chunks))
        statp = ctx.enter_context(tc.tile_pool(name="stat", bufs=1))
        sums = statp.tile([128, n_chunks], F32)
        sqs = statp.tile([128, n_chunks], F32)
        sqscr = statp.tile([128, QW], F32)
        xchs = []
        for n2 in range(nhalf):
            for q in range(NQ):
                xch = xchp.tile([128, QW], F32, tag="xch")
                xchs.append(xch)
                for half in range(2):
                    n = half * nhalf + n2
                    nc.sync.dma_start(
                        out=xch[half * C : half * C + C, :].rearrange(
                            "c (h w) -> c h w", w=W
                        ),
                        in_=x.ap()[n, :, q * Q_ROWS : (q + 1) * Q_ROWS, :],
                    )
        for idx, xch in enumerate(xchs):
            nc.vector.tensor_reduce(
                out=sums[:, idx : idx + 1], in_=xch,
                axis=mybir.AxisListType.X, op=mybir.AluOpType.add,
            )
            nc.scalar.activation(
                out=sqscr, in_=xch,
                func=mybir.ActivationFunctionType.Square,
                accum_out=sqs[:, idx : idx + 1],
            )
        res = statp.tile([128, 2], F32)
        nc.vector.tensor_reduce(
            out=res[:, 0:1], in_=sums,
            axis=mybir.AxisListType.X, op=mybir.AluOpType.add,
        )
        nc.vector.tensor_reduce(
            out=res[:, 1:2], in_=sqs,
            axis=mybir.AxisListType.X, op=mybir.AluOpType.add,
        )
        nc.sync.dma_start(out=s_out.ap(), in_=res)

    nc.compile()
    return nc


def build_conv_program(n_cores=N_CORES, n_img=N_IMG):
    """k2: binarize (thresholds given) + conv + relu, streaming x."""
    nhalf = n_img // 2
    nc = bacc.Bacc(
        "TRN2", target_bir_lowering=False, debug=False, num_devices=n_cores
    )
    x = nc.dram_tensor("x", [n_img, C, H, W], F32, kind="ExternalInput")
    Wt = nc.dram_tensor("W", [C, C, 3, 3], F32, kind="ExternalInput")
    bt = nc.dram_tensor("b", [C], F32, kind="ExternalInput")
    av = nc.dram_tensor("avec", [C], F32, kind="ExternalInput")
    dv = nc.dram_tensor("dvec", [C], F32, kind="ExternalInput")
    y = nc.dram_tensor("y", [n_img, C, H, W], F32, kind="ExternalOutput")

    with tile.TileContext(nc) as tc, ExitStack() as ctx:
        const = ctx.enter_context(tc.tile_pool(name="const", bufs=1))
        n_chunks = nhalf * NQ
        xchp = ctx.enter_context(tc.tile_pool(name="xch", bufs=n_chunks - 3))
        tmpp = ctx.enter_context(tc.tile_pool(name="tmpb", bufs=2))
        xbp = ctx.enter_context(tc.tile_pool(name="xb", bufs=4))
        psump = ctx.enter_context(tc.tile_pool(name="ps", bufs=6, space="PSUM"))
        pstr = ctx.enter_context(tc.tile_pool(name="pst", bufs=2, space="PSUM"))
        outp = ctx.enter_context(tc.tile_pool(name="out", bufs=8))

        identity64 = const.tile([C, C], F32)
        make_identity(nc, identity64)

        # all x chunk loads up front on the sync queue
        xchs = {}
        for n2 in range(nhalf):
            for q in range(NQ):
                xch = xchp.tile([128, QW], F32, tag="xch")
                xchs[(n2, q)] = xch
                for half in range(2):
                    n = half * nhalf + n2
                    nc.sync.dma_start(
                        out=xch[half * C : half * C + C, :].rearrange(
                            "c (h w) -> c h w", w=W
                        ),
                        in_=x.ap()[n, :, q * Q_ROWS : (q + 1) * Q_ROWS, :],
                    )

        # const DMAs on gpsimd
        wsb = const.tile([C, C, 9], F32)
        nc.gpsimd.dma_start(
            out=wsb, in_=Wt.ap().rearrange("o c kh kw -> o c (kh kw)")
        )
        b2 = const.tile([128, 1], F32)
        bsrc = bt.ap().rearrange("(c u) -> c u", u=1)
        nc.gpsimd.dma_start(out=b2[0:C, :], in_=bsrc)
        nc.gpsimd.dma_start(out=b2[C:128, :], in_=bsrc)
        a2 = const.tile([128, 1], F32)
        asrc = av.ap().rearrange("(c u) -> c u", u=1)
        nc.gpsimd.dma_start(out=a2[0:C, :], in_=asrc)
        nc.gpsimd.dma_start(out=a2[C:128, :], in_=asrc)
        d2 = const.tile([128, 1], F32)
        dsrc = dv.ap().rearrange("(c u) -> c u", u=1)
        nc.gpsimd.dma_start(out=d2[0:C, :], in_=dsrc)
        nc.gpsimd.dma_start(out=d2[C:128, :], in_=dsrc)

        # fp16 weight views via PE transposes
        w2 = const.tile([128, 9, C], WDT)
        for t in range(9):
            psT = pstr.tile([C, C], F32, tag="pst")
            nc.tensor.transpose(psT, wsb[:, :, t], identity64)
            nc.scalar.activation(
                out=w2[0:C, t, :], in_=psT,
                func=mybir.ActivationFunctionType.Copy,
            )
            if t >= 3:
                nc.scalar.activation(
                    out=w2[C:128, t - 3, :], in_=psT,
                    func=mybir.ActivationFunctionType.Copy,
                )

        out_engines = (nc.sync, nc.scalar)
        state = {"dma": 0}

        def conv_slot(n, xbv, s):
            h0 = s * 2 * ROWS_PER_CHUNK
            h1 = h0 + ROWS_PER_CHUNK
            P = psump.tile([128, NMM], F32, tag="psum")
            mms = []
            for kw in range(3):
                for cg, hb in ((0, h0), (64, h1)):
                    mms.append((cg, hb, kw, True))
            for kw in range(3):
                for cg, hb in ((0, h0), (64, h1)):
                    mms.append((cg, hb, kw, False))
            cg_seen = set()
            cg_last = {cg: max(i for i, m in enumerate(mms) if m[0] == cg)
                       for cg in (0, 64)}
            for i, (cg, hb, kw, is_pair) in enumerate(mms):
                if is_pair:
                    lhsT = w2[:, kw, :]
                    rhs = xbv[:, hb : hb + ROWS_PER_CHUNK, kw : kw + W]
                else:
                    lhsT = w2[0:C, 6 + kw, :]
                    rhs = xbv[0:C, hb + 2 : hb + 2 + ROWS_PER_CHUNK,
                              kw : kw + W]
                nc.tensor.matmul(
                    P[cg : cg + C, :], lhsT, rhs,
                    start=(cg not in cg_seen), stop=(i == cg_last[cg]),
                    tile_position=(0, cg), skip_group_check=True,
                )
                cg_seen.add(cg)
            osb = outp.tile([128, NMM], F32, tag="osb")
            if s % 2 == 0:
                nc.scalar.activation(
                    out=osb, in_=P,
                    func=mybir.ActivationFunctionType.Relu, bias=b2,
                )
            else:
                nc.vector.tensor_scalar(
                    out=osb, in0=P, scalar1=b2, scalar2=0.0,
                    op0=mybir.AluOpType.add, op1=mybir.AluOpType.max,
                )
            ov = osb.rearrange("p (h w) -> p h w", w=W)
            e0 = out_engines[state["dma"] % 2]
            e1 = out_engines[(state["dma"] + 1) % 2]
            state["dma"] += 2
            e0.dma_start(
                out=y.ap()[n, :, h0 : h0 + ROWS_PER_CHUNK, :],
                in_=ov[0:C, :, :],
            )
            e1.dma_start(
                out=y.ap()[n, :, h1 : h1 + ROWS_PER_CHUNK, :],
                in_=ov[C:128, :, :],
            )

        # stream: per image-pair, per chunk: fused Sign -> distribute ->
        # row-shifted copy -> conv slots as rows become available
        slot_hi = [(Q_ROWS * (q + 1) - 9) // 8 for q in range(NQ)]
        slot_hi[-1] = N_SLOTS - 1
        for n2 in range(nhalf):
            imgs = (n2, nhalf + n2)
            xbts, xbvs = [], []
            for n in imgs:
                xbt = xbp.tile([128, IMG], WDT, tag="xb")
                xbv = xbt.rearrange("p (hp wp) -> p hp wp", wp=WP)
                xbts.append(xbt)
                xbvs.append(xbv)
                nc.gpsimd.memset(xbv[0:C, 0:1, :], 0.0)
                nc.gpsimd.memset(xbv[0:C, HP - 1 : HP, :], 0.0)
                nc.gpsimd.memset(xbv[0:C, 1 : HP - 1, 0:1], 0.0)
                nc.gpsimd.memset(xbv[0:C, 1 : HP - 1, WP - 1 : WP], 0.0)
            slot_done = [0, 0]
            for q in range(NQ):
                xch = xchs[(n2, q)]
                h0c = q * Q_ROWS
                h1c = (q + 1) * Q_ROWS
                tmpb = tmpp.tile([128, QW], WDT, tag="tmpb")
                nc.scalar.activation(
                    out=tmpb, in_=xch,
                    func=mybir.ActivationFunctionType.Sign,
                    scale=a2, bias=d2,
                )
                for half in range(2):
                    nc.vector.tensor_copy(
                        out=xbvs[half][0:C, 1 + h0c : 1 + h1c, 1 : WP - 1],
                        in_=tmpb[half * C : half * C + C, :].rearrange(
                            "c (h w) -> c h w", w=W
                        ),
                    )
                    lo = 0 if q == 0 else (h0c - 1) * WP
                    hi_ = (h1c - 1) * WP if h1c < H else IMG - WP
                    nc.vector.tensor_copy(
                        out=xbts[half][C:128, lo:hi_],
                        in_=xbts[half][0:C, lo + WP : hi_ + WP],
                    )
                for half in range(2):
                    for s in range(slot_done[half], slot_hi[q] + 1):
                        conv_slot(imgs[half], xbvs[half], s)
                    slot_done[half] = slot_hi[q] + 1

    nc.compile()
    return nc


def _get_two_programs(n_cores=N_CORES, n_img=N_IMG):
    key = ("two", n_cores, n_img)
    if key not in _CACHE:
        _CACHE[key] = (
            build_stats_program(n_cores, n_img),
            build_conv_program(n_cores, n_img),
        )
    return _CACHE[key]


def kernel_two(x, gamma, beta, W, b, _trace=False):
    x = np.ascontiguousarray(x, dtype=np.float32)
    gamma = np.ascontiguousarray(gamma, np.float32)
    beta = np.ascontiguousarray(beta, np.float32)
    W = np.ascontiguousarray(W, np.float32)
    b = np.ascontiguousarray(b, np.float32)
    assert x.shape[0] == N_CORES * N_IMG, x.shape
    nc1, nc2 = _get_two_programs(N_CORES, N_IMG)
    shards = [x[c * N_IMG : (c + 1) * N_IMG] for c in range(N_CORES)]
    res1 = run_bass_kernel_spmd(
        nc1, [{"x": s} for s in shards],
        core_ids=list(range(N_CORES)), trace=_trace,
    )
    parts = np.stack([res1.results[c]["s_out"] for c in range(N_CORES)])
    tot = parts.astype(np.float64).sum(axis=0)
    tot64 = tot[:C] + tot[C:]
    count = float(N_CORES * N_IMG * PIX)
    mean = tot64[:, 0] / count
    var = tot64[:, 1] / count - mean * mean
    sigma = np.sqrt(var + EPS)
    avec = gamma.astype(np.float64)
    dvec = (beta.astype(np.float64) * sigma - avec * mean).astype(np.float32)
    avec = avec.astype(np.float32)
    res2 = run_bass_kernel_spmd(
        nc2,
        [{"x": s, "W": W, "b": b, "avec": avec, "dvec": dvec} for s in shards],
        core_ids=list(range(N_CORES)), trace=_trace,
    )
    out = np.concatenate([res2.results[c]["y"] for c in range(N_CORES)], axis=0)
    if _trace:
        kernel._last_result = (res1, res2)
    return out
